# revision 9
# baseline (speedup 1.0000x reference)
"""CNN+LSTM recognizer on 8 Trainium2 NeuronCores.

Data-parallel over the batch axis (8 samples per core, zero cross-core
communication). All weights are replicated; each core runs conv -> maxpool
-> pre-gate matmul (the x @ w_ih.T part of every LSTM step, batched over
time) -> the 512-step recurrence -> MLP head -> log_softmax on its batch
shard.

Recurrence design notes:
- w_hh is stored as fp8 e4m3 (x256 scale) so LDWEIGHTS runs at the 4x
  fast-weight-load rate; the descale folds into the existing
  scalar_tensor_tensor that adds the precomputed input gates.
- Gate tiles are grouped into two hidden-chunk halves (A = chunks 0..3,
  B = 4..7). Each step issues matmuls as [A|B @ k0..3], [A @ k4..7],
  chainA, [B @ k4..7], chainB, so each half's sigmoid/tanh/cell-update
  chain executes while the tensor engine streams the other half's
  matmuls, and the next step's k0..3 matmuls only need chainA's output.
"""

import os
import sys

sys.path.insert(0, "/opt/trn_rl_repo")

import json as _json

import ml_dtypes
import numpy as np

# ---------------------------------------------------------------- constants
S, B, D = 512, 64, 120
OC, KW = 16, 6
AFTER_CONV = (D - KW) + 1          # 115
AFTER_POOL = AFTER_CONV - 1        # 114
NF = OC * AFTER_POOL               # 1824 LSTM input features
NFP = 1920                         # padded to 15 * 128
H, O = 1024, 48
G4 = 4 * H                         # 4096 gate rows
N_CORES = 8
BL = B // N_CORES                  # 8 samples per core
NROWS = S * BL                     # 4096 (s, b) rows per core
KT = NFP // 128                    # 15 k-tiles for pre-gates
MT = G4 // 128                     # 32 gate m-tiles
HK = H // 128                      # 8 hidden chunks
N_STEPS = int(os.environ.get("BASS_LSTM_STEPS", str(S)))
PHASES = int(os.environ.get("BASS_PHASES", "3"))
WHH_SCALE = 256.0
# N=512 dummy matmuls issued per LSTM step to keep the PE HAM clock-gate
# released (the real FD=8 matmuls alone leave the PE array duty cycle so low
# that HAM throttles the PE clock to 1.2 GHz for the whole recurrence).
HAM_FEED = int(os.environ.get("BASS_HAM_FEED", "2"))

# gate-tile order: two halves of hidden chunks, gate-type major inside each:
#   m' in [0,16):  half A (chunks 0..3),  m' = gt*4 + c        (c in 0..3)
#   m' in [16,32): half B (chunks 4..7),  m' = 16 + gt*4 + (c-4)
# gt: 0=i, 1=f, 2=o, 3=g (torch rows i,f,g,o -> bases 0,H,3H,2H)
_GATE_BASE = [0, H, 3 * H, 2 * H]


def _gate_rows(m):
    if m < 16:
        gt, c = m // 4, m % 4
    else:
        gt, c = (m - 16) // 4, 4 + (m - 16) % 4
    base = _GATE_BASE[gt] + c * 128
    return np.arange(base, base + 128)


# ---------------------------------------------------------------- harness patches
def _install_patches():
    from concourse import tile
    import concourse.mybir as mybir
    import concourse.bass_utils as _bu
    import concourse.bass2jax as _b2j
    from concourse.vector_clock import ScopedClock

    if getattr(_bu, "_ant_lstm_patched", False):
        return

    def _patched_dab(self, tick_clock, wait_clock):
        # This walrus rejects >2 sem waits on one instruction; the tile tail
        # drain waits on every ticked proc. Spread waits over nop carriers.
        nc = self.nc
        carrier = nc.sync.nop(nofuse=True)
        wait_clock.add_sem_waits(
            carrier.ins, ScopedClock({None: tick_clock.global_clock})
        )
        si = carrier.ins.sync_info
        if si is not None and si.on_wait and len(si.on_wait) > 1:
            waits = list(si.on_wait)
            si.on_wait = waits[:1]
            for w in waits[1:]:
                extra = nc.sync.nop(nofuse=True)
                extra.ins.sync_info = mybir.SyncInfo(on_wait=[w], on_update=[])
        nc.sync.drain()
        nc.all_engine_barrier()
        popped = nc._tile_sem_poison_stack.pop()
        assert popped is self._sem_poison
        nc.clear_and_free_semaphores(list(self.sems.allocated().values()))
        nc.all_engine_barrier()

    tile.TileContext._drain_and_barrier = _patched_dab

    _MAXW = 1
    _orig_compile_bir = _bu.compile_bir_kernel

    def _split_excess_waits(bir_json: bytes) -> bytes:
        m = _json.loads(bir_json)
        changed = False
        for fn in m.get("functions", []):
            for blk in fn.get("blocks", []):
                insts = blk.get("instructions")
                if not insts:
                    continue
                out = []
                for i in insts:
                    si = i.get("sync_info")
                    ow = (si or {}).get("on_wait") or []
                    if len(ow) > _MAXW:
                        changed = True
                        extra, keep = ow[:-_MAXW], ow[-_MAXW:]
                        for k in range(0, len(extra), _MAXW):
                            out.append({
                                "debug": i.get("debug", 0),
                                "engine": i["engine"],
                                "ins": [], "outs": [],
                                "name": i["name"] + "_w%d" % k,
                                "opcode": "NoOp",
                                "sync_info": {"on_update": [],
                                              "on_wait": extra[k:k + _MAXW]},
                            })
                        si["on_wait"] = keep
                    out.append(i)
                blk["instructions"] = out
        return _json.dumps(m).encode() if changed else bir_json

    def _patched_compile_bir(bir_json, tmpdir, neff_name="file.neff"):
        return _orig_compile_bir(_split_excess_waits(bir_json), tmpdir, neff_name)

    _bu.compile_bir_kernel = _patched_compile_bir
    _b2j.compile_bir_kernel = _patched_compile_bir
    _bu._ant_lstm_patched = True


# ---------------------------------------------------------------- program
def _build_program():
    from concourse import bass, tile
    import concourse.mybir as mybir

    dt = mybir.dt
    AF = mybir.ActivationFunctionType
    ALU = mybir.AluOpType

    nc = bass.Bass()

    # ---- kernel I/O (per-core shards, host-prepared layouts)
    xt = nc.declare_dram_parameter("xt", [D, NROWS], dt.bfloat16, isOutput=False)
    w2a = nc.declare_dram_parameter("w2a", [D, KT, 128], dt.bfloat16, isOutput=False)
    w2b = nc.declare_dram_parameter("w2b", [D, KT, 128], dt.bfloat16, isOutput=False)
    bias_q = nc.declare_dram_parameter("bias_q", [128, KT], dt.float32, isOutput=False)
    wih_t = nc.declare_dram_parameter("wih_t", [MT, 128, KT, 128], dt.bfloat16, isOutput=False)
    bih_t = nc.declare_dram_parameter("bih_t", [128, MT], dt.float32, isOutput=False)
    bhh_t = nc.declare_dram_parameter("bhh_t", [128, MT], dt.float32, isOutput=False)
    whh_t = nc.declare_dram_parameter("whh_t", [128, HK, MT, 128], dt.float8e4, isOutput=False)
    h0t = nc.declare_dram_parameter("h0t", [128, HK, BL], dt.float32, isOutput=False)
    c0t = nc.declare_dram_parameter("c0t", [128, HK, BL], dt.float32, isOutput=False)
    h2h_tt = nc.declare_dram_parameter("h2h_t", [HK, 128, HK, 128], dt.bfloat16, isOutput=False)
    h2b_t = nc.declare_dram_parameter("h2b_t", [128, HK], dt.float32, isOutput=False)
    outw_t = nc.declare_dram_parameter("outw_t", [128, HK, O], dt.bfloat16, isOutput=False)
    outb_t = nc.declare_dram_parameter("outb_t", [1, O], dt.bfloat16, isOutput=False)
    ident_in = nc.declare_dram_parameter("ident", [128, 128], dt.bfloat16, isOutput=False)
    out_d = nc.declare_dram_parameter("out", [NROWS, O], dt.float32, isOutput=True)

    # internal scratch: pre-gates for every (t, b), step-sliceable
    gxt = nc.dram_tensor("gxt", [MT, 128, S, BL], dt.bfloat16)

    NCH1 = NROWS // 512            # 8 column chunks of 512 in phase 1

    with tile.TileContext(nc) as tc:
        cpool = tc.alloc_tile_pool(name="const", bufs=1)
        ident = cpool.tile([128, 128], dt.bfloat16)
        nc.sync.dma_start(ident[:], ident_in[:])
        biasq_sb = cpool.tile([128, KT], dt.float32)
        nc.sync.dma_start(biasq_sb[:], bias_q[:])
        bg_sb = cpool.tile([128, MT], dt.float32)   # b_ih + b_hh
        bih_sb = cpool.tile([128, MT], dt.float32)
        bhh_sb = cpool.tile([128, MT], dt.float32)
        nc.sync.dma_start(bih_sb[:], bih_t[:])
        nc.sync.dma_start(bhh_sb[:], bhh_t[:])
        nc.vector.tensor_add(bg_sb[:], bih_sb[:], bhh_sb[:])
        h2b_sb = cpool.tile([128, HK], dt.float32)
        nc.sync.dma_start(h2b_sb[:], h2b_t[:])
        outw_sb = cpool.tile([128, HK, O], dt.bfloat16)
        nc.sync.dma_start(outw_sb[:], outw_t[:])
        outb_sb = cpool.tile([1, O], dt.bfloat16)
        nc.sync.dma_start(outb_sb[:], outb_t[:])
        ones_sb = cpool.tile([1, 128], dt.bfloat16)
        nc.vector.memset(ones_sb[:], 1.0)

        # ---------------- phase 1: conv + maxpool + pre-gates -> gxt
        with (
            tc.tile_pool(name="xtp", bufs=1) as xtp,
            tc.tile_pool(name="featp", bufs=1) as featp,
        ):
            xt_sb = xtp.tile([D, NROWS], dt.bfloat16)
            nc.sync.dma_start(xt_sb[:], xt[:])
            feat = featp.tile([128, KT, NROWS], dt.bfloat16)

            with nc.named_scope("p1_conv"):
                with (
                    tc.tile_pool(name="w2p", bufs=1) as w2p,
                    tc.tile_pool(name="psc", bufs=2, space="PSUM") as pscp,
                    tc.tile_pool(name="mx1", bufs=4) as mx1p,
                ):
                    w2a_sb = w2p.tile([D, KT, 128], dt.bfloat16)
                    w2b_sb = w2p.tile([D, KT, 128], dt.bfloat16)
                    nc.sync.dma_start(w2a_sb[:], w2a[:])
                    nc.sync.dma_start(w2b_sb[:], w2b[:])
                    for nch in range(NCH1):
                        cs = slice(nch * 512, (nch + 1) * 512)
                        for m in range(KT):
                            pa = pscp.tile([128, 512], dt.float32, tag="psA")
                            pb = pscp.tile([128, 512], dt.float32, tag="psB")
                            nc.tensor.matmul(pa[:], w2a_sb[:, m, :], xt_sb[:, cs],
                                             start=True, stop=True)
                            nc.tensor.matmul(pb[:], w2b_sb[:, m, :], xt_sb[:, cs],
                                             start=True, stop=True)
                            pbs = mx1p.tile([128, 512], dt.float32, tag="pbs")
                            nc.scalar.activation(pbs[:], pb[:], AF.Identity)
                            mx = mx1p.tile([128, 512], dt.float32)
                            nc.vector.tensor_max(mx[:], pa[:], pbs[:])
                            nc.scalar.activation(feat[:, m, cs], mx[:], AF.Relu,
                                                 bias=biasq_sb[:, m:m + 1])

            # pre-gates: gxt[m][p, t, b] = (w_ih @ feat)[gate row, (t, b)] + bias
            with nc.named_scope("p1_pregate"):
                with (
                    tc.tile_pool(name="wihp", bufs=3) as wihp,
                    tc.tile_pool(name="ps1", bufs=8, space="PSUM") as ps1p,
                    tc.tile_pool(name="gst", bufs=2) as gstp,
                ):
                    for m in range(MT):
                        wt = wihp.tile([128, KT, 128], dt.bfloat16)
                        nc.sync.dma_start(wt[:], wih_t[m])
                        pss = [ps1p.tile([128, 64, BL], dt.float32, tag="psG",
                                         name="psg%d" % n)
                               for n in range(NCH1)]
                        for k in range(KT):
                            for nch in range(NCH1):
                                cs = slice(nch * 512, (nch + 1) * 512)
                                nc.tensor.matmul(pss[nch][:], wt[:, k, :], feat[:, k, cs],
                                                 start=(k == 0), stop=(k == KT - 1))
                        gs = gstp.tile([128, NCH1, 64, BL], dt.bfloat16)
                        for nch in range(NCH1):
                            nc.scalar.activation(gs[:, nch, :, :], pss[nch][:], AF.Identity,
                                                 bias=bg_sb[:, m:m + 1])
                        nc.sync.dma_start(gxt[m], gs[:])

        # ---------------- phase 2: LSTM recurrence
        if PHASES < 2:
            cpool.release()
            return nc
        arch_pool = tc.alloc_tile_pool(name="arch", bufs=1)
        arch = arch_pool.tile([128, HK, S, BL], dt.bfloat16)

        with nc.named_scope("p2_lstm"):
            with (
                tc.tile_pool(name="whhp", bufs=1) as whhp,
                tc.tile_pool(name="state", bufs=4) as stp,
                tc.tile_pool(name="gxp", bufs=3) as gxp,
                tc.tile_pool(name="ps2", bufs=3, space="PSUM") as ps2p,
            ):
                whh_sb = whhp.tile([128, HK, MT, 128], dt.float8e4)
                nc.sync.dma_start(whh_sb[:], whh_t[:])

                hamf_mv = whhp.tile([128, 512], dt.bfloat16)
                nc.vector.memset(hamf_mv[:], 0.0)
                hamf_ps = None
                if HAM_FEED:
                    hamf_pool = tc.alloc_tile_pool(name="hamf", bufs=1,
                                                   space="PSUM")
                    hamf_ps = hamf_pool.tile([128, 512], dt.float32)

                def _ham_feed():
                    # dep-free full-width matmul; result never read
                    nc.tensor.matmul(hamf_ps[:], ident[:], hamf_mv[:],
                                     start=True, stop=True,
                                     skip_group_check=True)

                h0_sb = stp.tile([128, HK, BL], dt.float32, tag="h0")
                c0_sb = stp.tile([128, HK, BL], dt.float32, tag="c0")
                nc.sync.dma_start(h0_sb[:], h0t[:])
                nc.sync.dma_start(c0_sb[:], c0t[:])
                hA = stp.tile([128, 4, BL], dt.bfloat16, tag="hA")
                hB = stp.tile([128, 4, BL], dt.bfloat16, tag="hB")
                cA = stp.tile([128, 4, BL], dt.float32, tag="cA")
                cB = stp.tile([128, 4, BL], dt.float32, tag="cB")
                nc.vector.tensor_copy(hA[:], h0_sb[:, 0:4, :])
                nc.vector.tensor_copy(hB[:], h0_sb[:, 4:8, :])
                nc.vector.tensor_copy(cA[:], c0_sb[:, 0:4, :])
                nc.vector.tensor_copy(cB[:], c0_sb[:, 4:8, :])

                GXB = 32
                NBLK = (N_STEPS + GXB - 1) // GXB
                gxblks = {}

                def _prefetch(blk):
                    if blk >= NBLK:
                        return
                    gb = gxp.tile([128, MT, GXB, BL], dt.bfloat16, tag="gx")
                    t0 = blk * GXB
                    for m in range(MT):
                        nc.sync.dma_start(gb[:, m, :, :], gxt[m][:, t0:t0 + GXB, :])
                    gxblks[blk] = gb

                _prefetch(0)
                _prefetch(1)

                DS = 1.0 / WHH_SCALE

                def _chain(ps, gx_ap, c_prev, lo, hi, t, htag, ctag):
                    # ps[:, 0] holds the k0..3 partial, ps[:, 1] the k4..7
                    # partial (separate accumulation groups so each group is
                    # contiguous within its psum zero region).
                    u = stp.tile([128, 16, BL], dt.float32, tag="u" + htag)
                    nc.vector.scalar_tensor_tensor(
                        u[:], ps[:, 0, :, :], DS, gx_ap, ALU.mult, ALU.add)
                    gsum = stp.tile([128, 16, BL], dt.float32, tag="gs" + htag)
                    nc.vector.scalar_tensor_tensor(
                        gsum[:], ps[:, 1, :, :], DS, u[:], ALU.mult, ALU.add)
                    sig = stp.tile([128, 12, BL], dt.float32, tag="sg" + htag)
                    nc.scalar.activation(sig[:], gsum[:, 0:12, :], AF.Sigmoid)
                    gg = stp.tile([128, 4, BL], dt.float32, tag="gg" + htag)
                    nc.scalar.activation(gg[:], gsum[:, 12:16, :], AF.Tanh)
                    t1 = stp.tile([128, 4, BL], dt.float32, tag="t1" + htag)
                    nc.vector.tensor_mul(t1[:], sig[:, 0:4, :], gg[:])
                    t2 = stp.tile([128, 4, BL], dt.float32, tag="t2" + htag)
                    nc.gpsimd.tensor_mul(t2[:], sig[:, 4:8, :], c_prev[:])
                    c_new = stp.tile([128, 4, BL], dt.float32, tag=ctag)
                    nc.vector.tensor_add(c_new[:], t1[:], t2[:])
                    tc_ = stp.tile([128, 4, BL], dt.float32, tag="tc" + htag)
                    nc.scalar.activation(tc_[:], c_new[:], AF.Tanh)
                    h_new = stp.tile([128, 4, BL], dt.bfloat16, tag=htag)
                    nc.vector.tensor_mul(h_new[:], sig[:, 8:12, :], tc_[:])
                    nc.scalar.activation(arch[:, lo:hi, t, :], h_new[:], AF.Relu)
                    return h_new, c_new

                for t in range(N_STEPS):
                    blk, off = divmod(t, GXB)
                    if off == 0 and t > 0:
                        _prefetch(blk + 1)
                        gxblks.pop(blk - 1, None)
                    gxb = gxblks[blk]
                    ps_A = ps2p.tile([128, 2, 16, BL], dt.float32, tag="psA")
                    ps_B = ps2p.tile([128, 2, 16, BL], dt.float32, tag="psB")
                    # k 0..3 for both halves (needs only hA from t-1); each
                    # mi's group is contiguous: start at k0, stop at k3.
                    for half, ps in ((0, ps_A), (1, ps_B)):
                        for mi in range(16):
                            m = half * 16 + mi
                            for k in range(4):
                                nc.tensor.matmul(ps[:, 0, mi, :],
                                                 whh_sb[:, k, m, :],
                                                 hA[:, k, :],
                                                 start=(k == 0), stop=(k == 3))
                    if HAM_FEED >= 2:
                        _ham_feed()
                    # A-half k 4..7 (needs hB from t-1)
                    for mi in range(16):
                        for k in range(4, 8):
                            nc.tensor.matmul(ps_A[:, 1, mi, :],
                                             whh_sb[:, k, mi, :],
                                             hB[:, k - 4, :],
                                             start=(k == 4), stop=(k == 7))
                    hA, cA = _chain(ps_A, gxb[:, 0:16, off, :], cA, 0, 4, t,
                                    "hA", "cA")
                    # B-half k 4..7
                    for mi in range(16):
                        m = 16 + mi
                        for k in range(4, 8):
                            nc.tensor.matmul(ps_B[:, 1, mi, :],
                                             whh_sb[:, k, m, :],
                                             hB[:, k - 4, :],
                                             start=(k == 4), stop=(k == 7))
                    if HAM_FEED >= 1:
                        _ham_feed()
                    hB, cB = _chain(ps_B, gxb[:, 16:32, off, :], cB, 4, 8, t,
                                    "hB", "cB")

                if hamf_ps is not None:
                    hamf_pool.release()

        # ---------------- phase 3: h2 = relu(hs @ h2h.T + b); logits; log_softmax
        if PHASES < 3:
            arch_pool.release()
            cpool.release()
            return nc
        with nc.named_scope("p3_head"):
            with tc.tile_pool(name="h2p", bufs=1) as h2p:
                NCH3 = N_STEPS * BL // 512 if N_STEPS * BL >= 512 else 1
                CW = min(512, N_STEPS * BL)
                h2_sb = h2p.tile([128, HK, NROWS], dt.bfloat16)
                with (
                    tc.tile_pool(name="h2hp", bufs=4) as h2hp,
                    tc.tile_pool(name="ps3", bufs=8, space="PSUM") as ps3p,
                ):
                    for m in range(HK):
                        wt = h2hp.tile([128, HK, 128], dt.bfloat16)
                        nc.sync.dma_start(wt[:], h2h_tt[m])
                        pss = [ps3p.tile([128, CW // BL, BL], dt.float32, tag="psH",
                                         name="ps3_%d" % n)
                               for n in range(NCH3)]
                        for k in range(HK):
                            for nch in range(NCH3):
                                ts = slice(nch * (CW // BL), (nch + 1) * (CW // BL))
                                nc.tensor.matmul(pss[nch][:], wt[:, k, :], arch[:, k, ts, :],
                                                 start=(k == 0), stop=(k == HK - 1))
                        for nch in range(NCH3):
                            cs = slice(nch * CW, (nch + 1) * CW)
                            nc.scalar.activation(h2_sb[:, m, cs], pss[nch][:], AF.Relu,
                                                 bias=h2b_sb[:, m:m + 1])

                with (
                    tc.tile_pool(name="ps4", bufs=4, space="PSUM") as ps4p,
                    tc.tile_pool(name="lsp", bufs=4) as lsp,
                ):
                    NRC = (N_STEPS * BL) // 128
                    for rc in range(NRC):
                        p4 = ps4p.tile([128, O], dt.float32)
                        rs = slice(rc * 128, (rc + 1) * 128)
                        for k in range(HK):
                            nc.tensor.matmul(p4[:], h2_sb[:, k, rs], outw_sb[:, k, :],
                                             start=(k == 0), stop=False,
                                             skip_group_check=True)
                        nc.tensor.matmul(p4[:], ones_sb[:], outb_sb[:],
                                         start=False, stop=True, skip_group_check=True)
                        mx = lsp.tile([128, 1], dt.float32, tag="mx")
                        nc.vector.tensor_reduce(mx[:], p4[:], mybir.AxisListType.X,
                                                mybir.AluOpType.max, negate=True)
                        ex = lsp.tile([128, O], dt.float32, tag="ex")
                        se = lsp.tile([128, 1], dt.float32, tag="se")
                        nc.scalar.activation(ex[:], p4[:], AF.Exp,
                                             bias=mx[:, 0:1], accum_out=se[:])
                        lnse = lsp.tile([128, 1], dt.float32, tag="ln")
                        nc.scalar.activation(lnse[:], se[:], AF.Ln)
                        shift = lsp.tile([128, 1], dt.float32, tag="sh")
                        nc.vector.tensor_sub(shift[:], mx[:], lnse[:])  # -max - ln(sum)
                        outt = lsp.tile([128, O], dt.float32, tag="out")
                        nc.vector.tensor_scalar_add(outt[:], p4[:], shift[:, 0:1])
                        nc.sync.dma_start(out_d[rs, :], outt[:])

        arch_pool.release()
        cpool.release()

    return nc


# ---------------------------------------------------------------- host side
def _bf(x):
    return np.asarray(x, np.float32).astype(ml_dtypes.bfloat16)


def _f8(x, scale):
    return (np.asarray(x, np.float32) * scale).astype(ml_dtypes.float8_e4m3)


def _prep_core_inputs(inputs, r):
    """Build in_maps[r] — pure layout transforms of the full inputs."""
    bs = slice(r * BL, (r + 1) * BL)
    x = np.asarray(inputs["input_"], np.float32)[:, bs, :]       # [S, BL, D]
    xt = np.ascontiguousarray(x.transpose(2, 0, 1).reshape(D, NROWS))

    conv_w = np.asarray(inputs["conv_w"], np.float32)            # [OC,1,KW]
    conv_b = np.asarray(inputs["conv_b"], np.float32)
    w2a = np.zeros((D, KT, 128), np.float32)
    w2b = np.zeros((D, KT, 128), np.float32)
    bias_q = np.zeros((128, KT), np.float32)
    for m in range(KT):
        for mc in range(128):
            q = m * 128 + mc
            if q >= NF:
                continue
            c, j = q // AFTER_POOL, q % AFTER_POOL
            w2a[j:j + KW, m, mc] = conv_w[c, 0, :]
            if j + 1 + KW <= D:
                w2b[j + 1:j + 1 + KW, m, mc] = conv_w[c, 0, :]
            bias_q[mc, m] = conv_b[c]

    w_ih = np.asarray(inputs["w_ih"], np.float32)                # [G4, NF]
    w_ih_p = np.zeros((G4, NFP), np.float32)
    w_ih_p[:, :NF] = w_ih
    wih_t = np.zeros((MT, 128, KT, 128), np.float32)
    rows_of = [_gate_rows(m) for m in range(MT)]
    for m in range(MT):
        blk = w_ih_p[rows_of[m], :]                              # [128, NFP]
        for k in range(KT):
            wih_t[m, :, k, :] = blk[:, k * 128:(k + 1) * 128].T
    w_hh = np.asarray(inputs["w_hh"], np.float32)                # [G4, H]
    whh_t = np.zeros((128, HK, MT, 128), np.float32)
    for m in range(MT):
        blk = w_hh[rows_of[m], :]
        for k in range(HK):
            whh_t[:, k, m, :] = blk[:, k * 128:(k + 1) * 128].T

    def _gvec(v):
        v = np.asarray(v, np.float32)
        out = np.zeros((128, MT), np.float32)
        for m in range(MT):
            out[:, m] = v[rows_of[m]]
        return out

    h2h_w = np.asarray(inputs["h2h_w"], np.float32)              # [H, H]
    h2h_t = np.zeros((HK, 128, HK, 128), np.float32)
    for m in range(HK):
        for k in range(HK):
            h2h_t[m, :, k, :] = h2h_w[m * 128:(m + 1) * 128, k * 128:(k + 1) * 128].T
    h2b = np.asarray(inputs["h2h_b"], np.float32).reshape(HK, 128).T.copy()

    out_w = np.asarray(inputs["out_w"], np.float32)              # [O, H]
    outw_t = np.ascontiguousarray(
        out_w.T.reshape(HK, 128, O).transpose(1, 0, 2))          # [128, HK, O]

    def _state_t(v):
        v = np.asarray(v, np.float32)[0, bs, :]                  # [BL, H]
        return np.ascontiguousarray(
            v.T.reshape(HK, 128, BL).transpose(1, 0, 2))         # [128, HK, BL]

    return {
        "xt": _bf(xt),
        "w2a": _bf(w2a), "w2b": _bf(w2b), "bias_q": bias_q,
        "wih_t": _bf(wih_t),
        "bih_t": _gvec(inputs["b_ih"]), "bhh_t": _gvec(inputs["b_hh"]),
        "whh_t": _f8(whh_t, WHH_SCALE),
        "h0t": _state_t(inputs["hidden"]), "c0t": _state_t(inputs["cell"]),
        "h2h_t": _bf(h2h_t), "h2b_t": h2b,
        "outw_t": _bf(outw_t), "outb_t": _bf(np.asarray(inputs["out_b"],
                                                        np.float32)[None, :]),
        "ident": _bf(np.eye(128, dtype=np.float32)),
    }


_CACHE = {}


def kernel(**inputs) -> np.ndarray:
    _install_patches()
    from concourse.bass_utils import run_bass_kernel_spmd

    if "nc" not in _CACHE:
        _CACHE["nc"] = _build_program()
    nc = _CACHE["nc"]

    in_maps = [_prep_core_inputs(inputs, r) for r in range(N_CORES)]
    res = run_bass_kernel_spmd(nc, in_maps, list(range(N_CORES)),
                               trace=bool(os.environ.get("BASS_TRACE_RUN")))
    _CACHE["last_result"] = res

    out = np.zeros((S, B, O), np.float32)
    for r in range(N_CORES):
        o = res.results[r]["out"].reshape(S, BL, O)
        out[:, r * BL:(r + 1) * BL, :] = o
    return out


# revision 21
# speedup vs baseline: 1.1741x; 1.1741x over previous
"""CNN+LSTM recognizer on 8 Trainium2 NeuronCores.

Data-parallel over the batch axis (8 samples per core, zero cross-core
communication). All weights are replicated; each core runs conv -> maxpool
-> pre-gate matmul (the x @ w_ih.T part of every LSTM step, batched over
time) -> the 512-step recurrence -> MLP head -> log_softmax on its batch
shard.

Recurrence design notes:
- w_hh is stored as fp8 e4m3 (x256 scale) so LDWEIGHTS runs at the 4x
  fast-weight-load rate; the descale folds into the existing
  scalar_tensor_tensor that adds the precomputed input gates.
- Gate tiles are grouped into two hidden-chunk halves (A = chunks 0..3,
  B = 4..7). Each step issues matmuls as [A|B @ k0..3], [A @ k4..7],
  chainA, [B @ k4..7], chainB, so each half's sigmoid/tanh/cell-update
  chain executes while the tensor engine streams the other half's
  matmuls, and the next step's k0..3 matmuls only need chainA's output.
"""

import os
import sys

sys.path.insert(0, "/opt/trn_rl_repo")

import json as _json

import ml_dtypes
import numpy as np

# ---------------------------------------------------------------- constants
S, B, D = 512, 64, 120
OC, KW = 16, 6
AFTER_CONV = (D - KW) + 1          # 115
AFTER_POOL = AFTER_CONV - 1        # 114
NF = OC * AFTER_POOL               # 1824 LSTM input features
NFP = 1920                         # padded to 15 * 128
H, O = 1024, 48
G4 = 4 * H                         # 4096 gate rows
N_CORES = 8
BL = B // N_CORES                  # 8 samples per core
NROWS = S * BL                     # 4096 (s, b) rows per core
KT = NFP // 128                    # 15 k-tiles for pre-gates
KT2 = 16                           # padded to even for fp8 DoubleRow pairs
F8S = 16.0                         # fp8 scale for feat / arch activations
W8S = 256.0                        # fp8 scale for w_ih / h2h weights
MT = G4 // 128                     # 32 gate m-tiles
HK = H // 128                      # 8 hidden chunks
N_STEPS = int(os.environ.get("BASS_LSTM_STEPS", str(S)))
PHASES = int(os.environ.get("BASS_PHASES", "3"))
WHH_SCALE = 256.0
# N=512 dummy matmuls issued per LSTM step to keep the PE HAM clock-gate
# released (the real FD=8 matmuls alone leave the PE array duty cycle so low
# that HAM throttles the PE clock to 1.2 GHz for the whole recurrence).
HAM_FEED = int(os.environ.get("BASS_HAM_FEED", "0"))

# gate-tile order: two halves of hidden chunks, gate-type major inside each:
#   m' in [0,16):  half A (chunks 0..3),  m' = gt*4 + c        (c in 0..3)
#   m' in [16,32): half B (chunks 4..7),  m' = 16 + gt*4 + (c-4)
# gt: 0=i, 1=f, 2=o, 3=g (torch rows i,f,g,o -> bases 0,H,3H,2H)
_GATE_BASE = [0, H, 3 * H, 2 * H]


def _gate_rows(m):
    if m < 16:
        gt, c = m // 4, m % 4
    else:
        gt, c = (m - 16) // 4, 4 + (m - 16) % 4
    base = _GATE_BASE[gt] + c * 128
    return np.arange(base, base + 128)


# ---------------------------------------------------------------- harness patches
def _install_patches():
    from concourse import tile
    import concourse.mybir as mybir
    import concourse.bass_utils as _bu
    import concourse.bass2jax as _b2j
    from concourse.vector_clock import ScopedClock

    if getattr(_bu, "_ant_lstm_patched", False):
        return

    def _patched_dab(self, tick_clock, wait_clock):
        # This walrus rejects >2 sem waits on one instruction; the tile tail
        # drain waits on every ticked proc. Spread waits over nop carriers.
        nc = self.nc
        carrier = nc.sync.nop(nofuse=True)
        wait_clock.add_sem_waits(
            carrier.ins, ScopedClock({None: tick_clock.global_clock})
        )
        si = carrier.ins.sync_info
        if si is not None and si.on_wait and len(si.on_wait) > 1:
            waits = list(si.on_wait)
            si.on_wait = waits[:1]
            for w in waits[1:]:
                extra = nc.sync.nop(nofuse=True)
                extra.ins.sync_info = mybir.SyncInfo(on_wait=[w], on_update=[])
        nc.sync.drain()
        nc.all_engine_barrier()
        popped = nc._tile_sem_poison_stack.pop()
        assert popped is self._sem_poison
        nc.clear_and_free_semaphores(list(self.sems.allocated().values()))
        nc.all_engine_barrier()

    tile.TileContext._drain_and_barrier = _patched_dab

    _MAXW = 1
    _orig_compile_bir = _bu.compile_bir_kernel

    def _split_excess_waits(bir_json: bytes) -> bytes:
        m = _json.loads(bir_json)
        changed = False
        for fn in m.get("functions", []):
            for blk in fn.get("blocks", []):
                insts = blk.get("instructions")
                if not insts:
                    continue
                out = []
                for i in insts:
                    si = i.get("sync_info")
                    ow = (si or {}).get("on_wait") or []
                    if len(ow) > _MAXW:
                        changed = True
                        extra, keep = ow[:-_MAXW], ow[-_MAXW:]
                        for k in range(0, len(extra), _MAXW):
                            out.append({
                                "debug": i.get("debug", 0),
                                "engine": i["engine"],
                                "ins": [], "outs": [],
                                "name": i["name"] + "_w%d" % k,
                                "opcode": "NoOp",
                                "sync_info": {"on_update": [],
                                              "on_wait": extra[k:k + _MAXW]},
                            })
                        si["on_wait"] = keep
                    out.append(i)
                blk["instructions"] = out
        return _json.dumps(m).encode() if changed else bir_json

    _DMAISH = ("DMA", "Trigger", "Collective")

    def _sparsify_sems(bir_json: bytes) -> bytes:
        """Drop per-instruction sem-inc updates nobody waits on.

        Tile ticks a per-engine semaphore on every instruction; the EVT_SEM
        register writes serialize at ~26 ns each, which caps the tensor
        engine at ~34 ns per matmul in the LSTM inner loop. Engine streams
        complete in program order, so a wait for "count >= v" is equivalent
        to a wait on the v-th updater alone. Keep an update only at awaited
        values and renumber waits by rank among kept updates.
        """
        m = _json.loads(bir_json)
        changed = False
        for fn in m.get("functions", []):
            upd_order = {}     # sem id -> [instruction update dicts in order]
            upd_owner = {}     # sem id -> set of engines
            bad = set()        # sems we must not touch
            waits = {}         # sem id -> set of awaited values
            for blk in fn.get("blocks", []):
                for i in blk.get("instructions", []) or []:
                    si = i.get("sync_info")
                    if not si:
                        continue
                    dma = any(s in i.get("opcode", "") for s in _DMAISH)
                    for u in si.get("on_update") or []:
                        sid = u.get("id")
                        if (u.get("sync_type") != "semaphore"
                                or u.get("update_mode") != "sem-inc"
                                or u.get("update_value") != 1 or dma):
                            bad.add(sid)
                        upd_order.setdefault(sid, []).append(u)
                        upd_owner.setdefault(sid, set()).add(i.get("engine"))
                    for w in si.get("on_wait") or []:
                        sid = w.get("id")
                        if (w.get("sync_type") != "semaphore"
                                or w.get("wait_mode") != "sem-ge-imm"):
                            bad.add(sid)
                        else:
                            waits.setdefault(sid, set()).add(w.get("wait_value"))
            for sid, owners in upd_owner.items():
                if len(owners) != 1:
                    bad.add(sid)
            # decide kept values per sem
            keep = {}
            for sid, ups in upd_order.items():
                if sid in bad:
                    continue
                awaited = sorted(v for v in waits.get(sid, set())
                                 if v is not None and v > 0)
                total = len(ups)
                if awaited and awaited[-1] > total:
                    continue  # unexpected; leave untouched
                keep[sid] = set(awaited)
            if not keep:
                continue
            # rewrite updates (pass 2)
            counters = {sid: 0 for sid in keep}
            for blk in fn.get("blocks", []):
                for i in blk.get("instructions", []) or []:
                    si = i.get("sync_info")
                    if not si:
                        continue
                    ou = si.get("on_update") or []
                    if ou:
                        new = []
                        for u in ou:
                            sid = u.get("id")
                            if sid in keep:
                                counters[sid] += 1
                                if counters[sid] in keep[sid]:
                                    new.append(u)
                                else:
                                    changed = True
                            else:
                                new.append(u)
                        si["on_update"] = new
                    for w in si.get("on_wait") or []:
                        sid = w.get("id")
                        if sid in keep:
                            v = w.get("wait_value")
                            if v and v > 0:
                                kept_vals = keep[sid]
                                w["wait_value"] = sum(
                                    1 for kv in kept_vals if kv <= v)
        return _json.dumps(m).encode() if changed else bir_json

    def _patched_compile_bir(bir_json, tmpdir, neff_name="file.neff"):
        return _orig_compile_bir(
            _split_excess_waits(_sparsify_sems(bir_json)), tmpdir, neff_name)

    _bu.compile_bir_kernel = _patched_compile_bir
    _b2j.compile_bir_kernel = _patched_compile_bir
    _bu._ant_lstm_patched = True


# ---------------------------------------------------------------- program
def _build_program():
    from concourse import bass, tile
    import concourse.mybir as mybir

    dt = mybir.dt
    AF = mybir.ActivationFunctionType
    ALU = mybir.AluOpType

    nc = bass.Bass()

    # ---- kernel I/O (per-core shards, host-prepared layouts)
    xt = nc.declare_dram_parameter("xt", [D, NROWS], dt.bfloat16, isOutput=False)
    w2a = nc.declare_dram_parameter("w2a", [D, KT, 128], dt.bfloat16, isOutput=False)
    w2b = nc.declare_dram_parameter("w2b", [D, KT, 128], dt.bfloat16, isOutput=False)
    bias_q = nc.declare_dram_parameter("bias_q", [128, KT], dt.float32, isOutput=False)
    wih_t = nc.declare_dram_parameter("wih_t", [MT, 128, KT2, 128], dt.float8e4, isOutput=False)
    bih_t = nc.declare_dram_parameter("bih_t", [128, MT], dt.float32, isOutput=False)
    bhh_t = nc.declare_dram_parameter("bhh_t", [128, MT], dt.float32, isOutput=False)
    whh_t = nc.declare_dram_parameter("whh_t", [128, HK, MT, 128], dt.float8e4, isOutput=False)
    h0t = nc.declare_dram_parameter("h0t", [128, HK, BL], dt.float32, isOutput=False)
    c0t = nc.declare_dram_parameter("c0t", [128, HK, BL], dt.float32, isOutput=False)
    h2h_tt = nc.declare_dram_parameter("h2h_t", [HK, 128, HK, 128], dt.float8e4, isOutput=False)
    h2b_t = nc.declare_dram_parameter("h2b_t", [128, HK], dt.float32, isOutput=False)
    outw_t = nc.declare_dram_parameter("outw_t", [128, HK, O], dt.bfloat16, isOutput=False)
    outb_t = nc.declare_dram_parameter("outb_t", [1, O], dt.bfloat16, isOutput=False)
    ident_in = nc.declare_dram_parameter("ident", [128, 128], dt.bfloat16, isOutput=False)
    out_d = nc.declare_dram_parameter("out", [NROWS, O], dt.float32, isOutput=True)

    # internal scratch: pre-gates for every (t, b), step-sliceable
    gxt = nc.dram_tensor("gxt", [MT, 128, S, BL], dt.bfloat16)

    NCH1 = NROWS // 512            # 8 column chunks of 512 in phase 1

    with tile.TileContext(nc) as tc:
        cpool = tc.alloc_tile_pool(name="const", bufs=1)
        ident = cpool.tile([128, 128], dt.bfloat16)
        nc.sync.dma_start(ident[:], ident_in[:])
        biasq_sb = cpool.tile([128, KT], dt.float32)
        nc.sync.dma_start(biasq_sb[:], bias_q[:])
        bg_sb = cpool.tile([128, MT], dt.float32)   # b_ih + b_hh
        bih_sb = cpool.tile([128, MT], dt.float32)
        bhh_sb = cpool.tile([128, MT], dt.float32)
        nc.sync.dma_start(bih_sb[:], bih_t[:])
        nc.sync.dma_start(bhh_sb[:], bhh_t[:])
        nc.vector.tensor_add(bg_sb[:], bih_sb[:], bhh_sb[:])
        h2b_sb = cpool.tile([128, HK], dt.float32)
        nc.sync.dma_start(h2b_sb[:], h2b_t[:])
        outw_sb = cpool.tile([128, HK, O], dt.bfloat16)
        nc.sync.dma_start(outw_sb[:], outw_t[:])
        outb_sb = cpool.tile([1, O], dt.bfloat16)
        nc.sync.dma_start(outb_sb[:], outb_t[:])
        ones_sb = cpool.tile([1, 128], dt.bfloat16)
        nc.vector.memset(ones_sb[:], 1.0)

        # ---------------- phase 1: conv + maxpool + pre-gates -> gxt
        with (
            tc.tile_pool(name="xtp", bufs=1) as xtp,
            tc.tile_pool(name="featp", bufs=1) as featp,
        ):
            xt_sb = xtp.tile([D, NROWS], dt.bfloat16)
            nc.sync.dma_start(xt_sb[:], xt[:])
            feat = featp.tile([128, KT2, NROWS], dt.float8e4)
            nc.vector.memset(feat[:, KT, :], 0.0)   # zero pad tile 15

            with nc.named_scope("p1_conv"):
                with (
                    tc.tile_pool(name="w2p", bufs=1) as w2p,
                    tc.tile_pool(name="psc", bufs=2, space="PSUM") as pscp,
                    tc.tile_pool(name="mx1", bufs=4) as mx1p,
                ):
                    w2a_sb = w2p.tile([D, KT, 128], dt.bfloat16)
                    w2b_sb = w2p.tile([D, KT, 128], dt.bfloat16)
                    nc.sync.dma_start(w2a_sb[:], w2a[:])
                    nc.sync.dma_start(w2b_sb[:], w2b[:])
                    for nch in range(NCH1):
                        cs = slice(nch * 512, (nch + 1) * 512)
                        for m in range(KT):
                            pa = pscp.tile([128, 512], dt.float32, tag="psA")
                            pb = pscp.tile([128, 512], dt.float32, tag="psB")
                            nc.tensor.matmul(pa[:], w2a_sb[:, m, :], xt_sb[:, cs],
                                             start=True, stop=True)
                            nc.tensor.matmul(pb[:], w2b_sb[:, m, :], xt_sb[:, cs],
                                             start=True, stop=True)
                            pbs = mx1p.tile([128, 512], dt.float32, tag="pbs")
                            nc.scalar.activation(pbs[:], pb[:], AF.Identity)
                            mx = mx1p.tile([128, 512], dt.float32)
                            nc.vector.tensor_max(mx[:], pa[:], pbs[:])
                            # bias_q is pre-scaled by F8S on the host, so this
                            # writes F8S * relu(conv + bias) into fp8 feat
                            nc.scalar.activation(feat[:, m, cs], mx[:], AF.Relu,
                                                 bias=biasq_sb[:, m:m + 1],
                                                 scale=F8S)

            # pre-gates: gxt[m][p, t, b] = (w_ih @ feat)[gate row, (t, b)] + bias
            with nc.named_scope("p1_pregate"):
                with (
                    tc.tile_pool(name="wihp", bufs=3) as wihp,
                    tc.tile_pool(name="ps1", bufs=8, space="PSUM") as ps1p,
                    tc.tile_pool(name="gst", bufs=2) as gstp,
                ):
                    for m in range(MT):
                        wt = wihp.tile([128, KT2, 128], dt.float8e4)
                        nc.sync.dma_start(wt[:], wih_t[m])
                        pss = [ps1p.tile([128, 64, BL], dt.float32, tag="psG",
                                         name="psg%d" % n)
                               for n in range(NCH1)]
                        for kk in range(KT2 // 2):
                            for nch in range(NCH1):
                                cs = slice(nch * 512, (nch + 1) * 512)
                                nc.tensor.matmul(
                                    pss[nch][:], wt[:, 2 * kk:2 * kk + 2, :],
                                    feat[:, 2 * kk:2 * kk + 2, cs],
                                    start=(kk == 0), stop=(kk == KT2 // 2 - 1),
                                    perf_mode=mybir.MatmulPerfMode.DoubleRow)
                        gs = gstp.tile([128, NCH1, 64, BL], dt.bfloat16)
                        for nch in range(NCH1):
                            nc.scalar.activation(gs[:, nch, :, :], pss[nch][:], AF.Identity,
                                                 bias=bg_sb[:, m:m + 1],
                                                 scale=1.0 / (F8S * W8S))
                        nc.sync.dma_start(gxt[m], gs[:])

        # ---------------- phase 2: LSTM recurrence
        if PHASES < 2:
            cpool.release()
            return nc
        arch_pool = tc.alloc_tile_pool(name="arch", bufs=1)
        arch = arch_pool.tile([128, HK, S, BL], dt.float8e4)

        with nc.named_scope("p2_lstm"):
            with (
                tc.tile_pool(name="whhp", bufs=1) as whhp,
                tc.tile_pool(name="state", bufs=4) as stp,
                tc.tile_pool(name="gxp", bufs=3) as gxp,
                tc.tile_pool(name="ps2", bufs=3, space="PSUM") as ps2p,
            ):
                whh_sb = whhp.tile([128, HK, MT, 128], dt.float8e4)
                nc.sync.dma_start(whh_sb[:], whh_t[:])

                hamf_mv = whhp.tile([128, 512], dt.bfloat16)
                nc.vector.memset(hamf_mv[:], 0.0)
                hamf_ps = None
                if HAM_FEED:
                    hamf_pool = tc.alloc_tile_pool(name="hamf", bufs=1,
                                                   space="PSUM")
                    hamf_ps = hamf_pool.tile([128, 512], dt.float32)

                def _ham_feed():
                    # dep-free full-width matmul; result never read
                    nc.tensor.matmul(hamf_ps[:], ident[:], hamf_mv[:],
                                     start=True, stop=True,
                                     skip_group_check=True)

                h0_sb = stp.tile([128, HK, BL], dt.float32, tag="h0")
                c0_sb = stp.tile([128, HK, BL], dt.float32, tag="c0")
                nc.sync.dma_start(h0_sb[:], h0t[:])
                nc.sync.dma_start(c0_sb[:], c0t[:])
                hA = stp.tile([128, 4, BL], dt.bfloat16, tag="hA")
                hB = stp.tile([128, 4, BL], dt.bfloat16, tag="hB")
                cA = stp.tile([128, 4, BL], dt.float32, tag="cA")
                cB = stp.tile([128, 4, BL], dt.float32, tag="cB")
                nc.vector.tensor_copy(hA[:], h0_sb[:, 0:4, :])
                nc.vector.tensor_copy(hB[:], h0_sb[:, 4:8, :])
                nc.vector.tensor_copy(cA[:], c0_sb[:, 0:4, :])
                nc.vector.tensor_copy(cB[:], c0_sb[:, 4:8, :])

                GXB = 32
                NBLK = (N_STEPS + GXB - 1) // GXB
                gxblks = {}

                def _prefetch(blk):
                    if blk >= NBLK:
                        return
                    gb = gxp.tile([128, MT, GXB, BL], dt.bfloat16, tag="gx")
                    t0 = blk * GXB
                    for m in range(MT):
                        nc.sync.dma_start(gb[:, m, :, :], gxt[m][:, t0:t0 + GXB, :])
                    gxblks[blk] = gb

                _prefetch(0)
                _prefetch(1)

                DS = 1.0 / WHH_SCALE

                def _chain(ps, gx_ap, c_prev, lo, hi, t, htag, ctag):
                    # ps[:, 0] holds the k0..3 partial, ps[:, 1] the k4..7
                    # partial (separate accumulation groups so each group is
                    # contiguous within its psum zero region).
                    u = stp.tile([128, 16, BL], dt.float32, tag="u" + htag)
                    nc.vector.scalar_tensor_tensor(
                        u[:], ps[:, 0, :, :], DS, gx_ap, ALU.mult, ALU.add)
                    gsum = stp.tile([128, 16, BL], dt.float32, tag="gs" + htag)
                    nc.vector.scalar_tensor_tensor(
                        gsum[:], ps[:, 1, :, :], DS, u[:], ALU.mult, ALU.add)
                    sig = stp.tile([128, 12, BL], dt.float32, tag="sg" + htag)
                    nc.scalar.activation(sig[:], gsum[:, 0:12, :], AF.Sigmoid)
                    gg = stp.tile([128, 4, BL], dt.float32, tag="gg" + htag)
                    nc.scalar.activation(gg[:], gsum[:, 12:16, :], AF.Tanh)
                    t1 = stp.tile([128, 4, BL], dt.float32, tag="t1" + htag)
                    nc.vector.tensor_mul(t1[:], sig[:, 0:4, :], gg[:])
                    t2 = stp.tile([128, 4, BL], dt.float32, tag="t2" + htag)
                    nc.gpsimd.tensor_mul(t2[:], sig[:, 4:8, :], c_prev[:])
                    c_new = stp.tile([128, 4, BL], dt.float32, tag=ctag)
                    nc.vector.tensor_add(c_new[:], t1[:], t2[:])
                    tc_ = stp.tile([128, 4, BL], dt.float32, tag="tc" + htag)
                    nc.scalar.activation(tc_[:], c_new[:], AF.Tanh)
                    h_new = stp.tile([128, 4, BL], dt.bfloat16, tag=htag)
                    nc.vector.tensor_mul(h_new[:], sig[:, 8:12, :], tc_[:])
                    nc.scalar.activation(arch[:, lo:hi, t, :], h_new[:], AF.Relu,
                                         scale=F8S)
                    return h_new, c_new

                for t in range(N_STEPS):
                    blk, off = divmod(t, GXB)
                    if off == 0 and t > 0:
                        _prefetch(blk + 1)
                        gxblks.pop(blk - 1, None)
                    gxb = gxblks[blk]
                    ps_A = ps2p.tile([128, 2, 16, BL], dt.float32, tag="psA")
                    ps_B = ps2p.tile([128, 2, 16, BL], dt.float32, tag="psB")
                    # k 0..3 for both halves (needs only hA from t-1); each
                    # mi's group is contiguous: start at k0, stop at k3.
                    for half, ps in ((0, ps_A), (1, ps_B)):
                        for mi in range(16):
                            m = half * 16 + mi
                            for k in range(4):
                                nc.tensor.matmul(ps[:, 0, mi, :],
                                                 whh_sb[:, k, m, :],
                                                 hA[:, k, :],
                                                 start=(k == 0), stop=(k == 3))
                    if HAM_FEED >= 2:
                        _ham_feed()
                    # A-half k 4..7 (needs hB from t-1)
                    for mi in range(16):
                        for k in range(4, 8):
                            nc.tensor.matmul(ps_A[:, 1, mi, :],
                                             whh_sb[:, k, mi, :],
                                             hB[:, k - 4, :],
                                             start=(k == 4), stop=(k == 7))
                    hA, cA = _chain(ps_A, gxb[:, 0:16, off, :], cA, 0, 4, t,
                                    "hA", "cA")
                    # B-half k 4..7
                    for mi in range(16):
                        m = 16 + mi
                        for k in range(4, 8):
                            nc.tensor.matmul(ps_B[:, 1, mi, :],
                                             whh_sb[:, k, m, :],
                                             hB[:, k - 4, :],
                                             start=(k == 4), stop=(k == 7))
                    if HAM_FEED >= 1:
                        _ham_feed()
                    hB, cB = _chain(ps_B, gxb[:, 16:32, off, :], cB, 4, 8, t,
                                    "hB", "cB")

                if hamf_ps is not None:
                    hamf_pool.release()

        # ---------------- phase 3: h2 = relu(hs @ h2h.T + b); logits; log_softmax
        if PHASES < 3:
            arch_pool.release()
            cpool.release()
            return nc
        with nc.named_scope("p3_head"):
            with tc.tile_pool(name="h2p", bufs=1) as h2p:
                NCH3 = N_STEPS * BL // 512 if N_STEPS * BL >= 512 else 1
                CW = min(512, N_STEPS * BL)
                h2_sb = h2p.tile([128, HK, NROWS], dt.bfloat16)
                with (
                    tc.tile_pool(name="h2hp", bufs=4) as h2hp,
                    tc.tile_pool(name="ps3", bufs=8, space="PSUM") as ps3p,
                ):
                    for m in range(HK):
                        wt = h2hp.tile([128, HK, 128], dt.float8e4)
                        nc.sync.dma_start(wt[:], h2h_tt[m])
                        pss = [ps3p.tile([128, CW // BL, BL], dt.float32, tag="psH",
                                         name="ps3_%d" % n)
                               for n in range(NCH3)]
                        for kk in range(HK // 2):
                            for nch in range(NCH3):
                                ts = slice(nch * (CW // BL), (nch + 1) * (CW // BL))
                                nc.tensor.matmul(
                                    pss[nch][:], wt[:, 2 * kk:2 * kk + 2, :],
                                    arch[:, 2 * kk:2 * kk + 2, ts, :],
                                    start=(kk == 0), stop=(kk == HK // 2 - 1),
                                    perf_mode=mybir.MatmulPerfMode.DoubleRow)
                        for nch in range(NCH3):
                            cs = slice(nch * CW, (nch + 1) * CW)
                            nc.scalar.activation(h2_sb[:, m, cs], pss[nch][:], AF.Relu,
                                                 bias=h2b_sb[:, m:m + 1],
                                                 scale=1.0 / (F8S * W8S))

                with (
                    tc.tile_pool(name="ps4", bufs=4, space="PSUM") as ps4p,
                    tc.tile_pool(name="lsp", bufs=4) as lsp,
                ):
                    NRC = (N_STEPS * BL) // 128
                    for rc in range(NRC):
                        p4 = ps4p.tile([128, O], dt.float32)
                        rs = slice(rc * 128, (rc + 1) * 128)
                        for k in range(HK):
                            nc.tensor.matmul(p4[:], h2_sb[:, k, rs], outw_sb[:, k, :],
                                             start=(k == 0), stop=False,
                                             skip_group_check=True)
                        nc.tensor.matmul(p4[:], ones_sb[:], outb_sb[:],
                                         start=False, stop=True, skip_group_check=True)
                        mx = lsp.tile([128, 1], dt.float32, tag="mx")
                        nc.vector.tensor_reduce(mx[:], p4[:], mybir.AxisListType.X,
                                                mybir.AluOpType.max, negate=True)
                        ex = lsp.tile([128, O], dt.float32, tag="ex")
                        se = lsp.tile([128, 1], dt.float32, tag="se")
                        nc.scalar.activation(ex[:], p4[:], AF.Exp,
                                             bias=mx[:, 0:1], accum_out=se[:])
                        lnse = lsp.tile([128, 1], dt.float32, tag="ln")
                        nc.scalar.activation(lnse[:], se[:], AF.Ln)
                        shift = lsp.tile([128, 1], dt.float32, tag="sh")
                        nc.vector.tensor_sub(shift[:], mx[:], lnse[:])  # -max - ln(sum)
                        outt = lsp.tile([128, O], dt.float32, tag="out")
                        nc.vector.tensor_scalar_add(outt[:], p4[:], shift[:, 0:1])
                        nc.sync.dma_start(out_d[rs, :], outt[:])

        arch_pool.release()
        cpool.release()

    return nc


# ---------------------------------------------------------------- host side
def _bf(x):
    return np.asarray(x, np.float32).astype(ml_dtypes.bfloat16)


def _f8(x, scale):
    return (np.asarray(x, np.float32) * scale).astype(ml_dtypes.float8_e4m3)


def _prep_core_inputs(inputs, r):
    """Build in_maps[r] — pure layout transforms of the full inputs."""
    bs = slice(r * BL, (r + 1) * BL)
    x = np.asarray(inputs["input_"], np.float32)[:, bs, :]       # [S, BL, D]
    xt = np.ascontiguousarray(x.transpose(2, 0, 1).reshape(D, NROWS))

    conv_w = np.asarray(inputs["conv_w"], np.float32)            # [OC,1,KW]
    conv_b = np.asarray(inputs["conv_b"], np.float32)
    w2a = np.zeros((D, KT, 128), np.float32)
    w2b = np.zeros((D, KT, 128), np.float32)
    bias_q = np.zeros((128, KT), np.float32)
    for m in range(KT):
        for mc in range(128):
            q = m * 128 + mc
            if q >= NF:
                continue
            c, j = q // AFTER_POOL, q % AFTER_POOL
            w2a[j:j + KW, m, mc] = conv_w[c, 0, :]
            if j + 1 + KW <= D:
                w2b[j + 1:j + 1 + KW, m, mc] = conv_w[c, 0, :]
            bias_q[mc, m] = conv_b[c] * F8S   # activation uses scale=F8S

    w_ih = np.asarray(inputs["w_ih"], np.float32)                # [G4, NF]
    w_ih_p = np.zeros((G4, NFP), np.float32)
    w_ih_p[:, :NF] = w_ih
    wih_t = np.zeros((MT, 128, KT, 128), np.float32)
    rows_of = [_gate_rows(m) for m in range(MT)]
    for m in range(MT):
        blk = w_ih_p[rows_of[m], :]                              # [128, NFP]
        for k in range(KT):
            wih_t[m, :, k, :] = blk[:, k * 128:(k + 1) * 128].T
    w_hh = np.asarray(inputs["w_hh"], np.float32)                # [G4, H]
    whh_t = np.zeros((128, HK, MT, 128), np.float32)
    for m in range(MT):
        blk = w_hh[rows_of[m], :]
        for k in range(HK):
            whh_t[:, k, m, :] = blk[:, k * 128:(k + 1) * 128].T

    def _gvec(v):
        v = np.asarray(v, np.float32)
        out = np.zeros((128, MT), np.float32)
        for m in range(MT):
            out[:, m] = v[rows_of[m]]
        return out

    h2h_w = np.asarray(inputs["h2h_w"], np.float32)              # [H, H]
    h2h_t = np.zeros((HK, 128, HK, 128), np.float32)
    for m in range(HK):
        for k in range(HK):
            h2h_t[m, :, k, :] = h2h_w[m * 128:(m + 1) * 128, k * 128:(k + 1) * 128].T
    h2b = np.asarray(inputs["h2h_b"], np.float32).reshape(HK, 128).T.copy()

    out_w = np.asarray(inputs["out_w"], np.float32)              # [O, H]
    outw_t = np.ascontiguousarray(
        out_w.T.reshape(HK, 128, O).transpose(1, 0, 2))          # [128, HK, O]

    def _state_t(v):
        v = np.asarray(v, np.float32)[0, bs, :]                  # [BL, H]
        return np.ascontiguousarray(
            v.T.reshape(HK, 128, BL).transpose(1, 0, 2))         # [128, HK, BL]

    return {
        "xt": _bf(xt),
        "w2a": _bf(w2a), "w2b": _bf(w2b), "bias_q": bias_q,
        "wih_t": _bf(wih_t),
        "bih_t": _gvec(inputs["b_ih"]), "bhh_t": _gvec(inputs["b_hh"]),
        "whh_t": _f8(whh_t, WHH_SCALE),
        "h0t": _state_t(inputs["hidden"]), "c0t": _state_t(inputs["cell"]),
        "h2h_t": _bf(h2h_t), "h2b_t": h2b,
        "outw_t": _bf(outw_t), "outb_t": _bf(np.asarray(inputs["out_b"],
                                                        np.float32)[None, :]),
        "ident": _bf(np.eye(128, dtype=np.float32)),
    }


_CACHE = {}


def kernel(**inputs) -> np.ndarray:
    _install_patches()
    from concourse.bass_utils import run_bass_kernel_spmd

    if "nc" not in _CACHE:
        _CACHE["nc"] = _build_program()
    nc = _CACHE["nc"]

    in_maps = [_prep_core_inputs(inputs, r) for r in range(N_CORES)]
    res = run_bass_kernel_spmd(nc, in_maps, list(range(N_CORES)),
                               trace=bool(os.environ.get("BASS_TRACE_RUN")))
    _CACHE["last_result"] = res

    out = np.zeros((S, B, O), np.float32)
    for r in range(N_CORES):
        o = res.results[r]["out"].reshape(S, BL, O)
        out[:, r * BL:(r + 1) * BL, :] = o
    return out


# revision 22
# speedup vs baseline: 1.2687x; 1.0806x over previous
"""CNN+LSTM recognizer on 8 Trainium2 NeuronCores.

Data-parallel over the batch axis (8 samples per core, zero cross-core
communication). All weights are replicated; each core runs conv -> maxpool
-> pre-gate matmul (the x @ w_ih.T part of every LSTM step, batched over
time) -> the 512-step recurrence -> MLP head -> log_softmax on its batch
shard.

Recurrence design notes:
- w_hh is stored as fp8 e4m3 (x256 scale) so LDWEIGHTS runs at the 4x
  fast-weight-load rate; the descale folds into the existing
  scalar_tensor_tensor that adds the precomputed input gates.
- Gate tiles are grouped into two hidden-chunk halves (A = chunks 0..3,
  B = 4..7). Each step issues matmuls as [A|B @ k0..3], [A @ k4..7],
  chainA, [B @ k4..7], chainB, so each half's sigmoid/tanh/cell-update
  chain executes while the tensor engine streams the other half's
  matmuls, and the next step's k0..3 matmuls only need chainA's output.
"""

import os
import sys

sys.path.insert(0, "/opt/trn_rl_repo")

import json as _json

import ml_dtypes
import numpy as np

# ---------------------------------------------------------------- constants
S, B, D = 512, 64, 120
OC, KW = 16, 6
AFTER_CONV = (D - KW) + 1          # 115
AFTER_POOL = AFTER_CONV - 1        # 114
NF = OC * AFTER_POOL               # 1824 LSTM input features
NFP = 1920                         # padded to 15 * 128
H, O = 1024, 48
G4 = 4 * H                         # 4096 gate rows
N_CORES = 8
BL = B // N_CORES                  # 8 samples per core
NROWS = S * BL                     # 4096 (s, b) rows per core
KT = NFP // 128                    # 15 k-tiles for pre-gates
MT = G4 // 128                     # 32 gate m-tiles
HK = H // 128                      # 8 hidden chunks
N_STEPS = int(os.environ.get("BASS_LSTM_STEPS", str(S)))
PHASES = int(os.environ.get("BASS_PHASES", "3"))
WHH_SCALE = 256.0
# N=512 dummy matmuls issued per LSTM step to keep the PE HAM clock-gate
# released (the real FD=8 matmuls alone leave the PE array duty cycle so low
# that HAM throttles the PE clock to 1.2 GHz for the whole recurrence).
HAM_FEED = int(os.environ.get("BASS_HAM_FEED", "0"))

# gate-tile order: two halves of hidden chunks, gate-type major inside each:
#   m' in [0,16):  half A (chunks 0..3),  m' = gt*4 + c        (c in 0..3)
#   m' in [16,32): half B (chunks 4..7),  m' = 16 + gt*4 + (c-4)
# gt: 0=i, 1=f, 2=o, 3=g (torch rows i,f,g,o -> bases 0,H,3H,2H)
_GATE_BASE = [0, H, 3 * H, 2 * H]


def _gate_rows(m):
    if m < 16:
        gt, c = m // 4, m % 4
    else:
        gt, c = (m - 16) // 4, 4 + (m - 16) % 4
    base = _GATE_BASE[gt] + c * 128
    return np.arange(base, base + 128)


# ---------------------------------------------------------------- harness patches
def _install_patches():
    from concourse import tile
    import concourse.mybir as mybir
    import concourse.bass_utils as _bu
    import concourse.bass2jax as _b2j
    from concourse.vector_clock import ScopedClock

    if getattr(_bu, "_ant_lstm_patched", False):
        return

    def _patched_dab(self, tick_clock, wait_clock):
        # This walrus rejects >2 sem waits on one instruction; the tile tail
        # drain waits on every ticked proc. Spread waits over nop carriers.
        nc = self.nc
        carrier = nc.sync.nop(nofuse=True)
        wait_clock.add_sem_waits(
            carrier.ins, ScopedClock({None: tick_clock.global_clock})
        )
        si = carrier.ins.sync_info
        if si is not None and si.on_wait and len(si.on_wait) > 1:
            waits = list(si.on_wait)
            si.on_wait = waits[:1]
            for w in waits[1:]:
                extra = nc.sync.nop(nofuse=True)
                extra.ins.sync_info = mybir.SyncInfo(on_wait=[w], on_update=[])
        nc.sync.drain()
        nc.all_engine_barrier()
        popped = nc._tile_sem_poison_stack.pop()
        assert popped is self._sem_poison
        nc.clear_and_free_semaphores(list(self.sems.allocated().values()))
        nc.all_engine_barrier()

    tile.TileContext._drain_and_barrier = _patched_dab

    _MAXW = 1
    _orig_compile_bir = _bu.compile_bir_kernel

    def _split_excess_waits(bir_json: bytes) -> bytes:
        m = _json.loads(bir_json)
        changed = False
        for fn in m.get("functions", []):
            for blk in fn.get("blocks", []):
                insts = blk.get("instructions")
                if not insts:
                    continue
                out = []
                for i in insts:
                    si = i.get("sync_info")
                    ow = (si or {}).get("on_wait") or []
                    if len(ow) > _MAXW:
                        changed = True
                        extra, keep = ow[:-_MAXW], ow[-_MAXW:]
                        for k in range(0, len(extra), _MAXW):
                            out.append({
                                "debug": i.get("debug", 0),
                                "engine": i["engine"],
                                "ins": [], "outs": [],
                                "name": i["name"] + "_w%d" % k,
                                "opcode": "NoOp",
                                "sync_info": {"on_update": [],
                                              "on_wait": extra[k:k + _MAXW]},
                            })
                        si["on_wait"] = keep
                    out.append(i)
                blk["instructions"] = out
        return _json.dumps(m).encode() if changed else bir_json

    _DMAISH = ("DMA", "Trigger", "Collective")

    def _sparsify_sems(bir_json: bytes) -> bytes:
        """Drop per-instruction sem-inc updates nobody waits on.

        Tile ticks a per-engine semaphore on every instruction; the EVT_SEM
        register writes serialize at ~26 ns each, which caps the tensor
        engine at ~34 ns per matmul in the LSTM inner loop. Engine streams
        complete in program order, so a wait for "count >= v" is equivalent
        to a wait on the v-th updater alone. Keep an update only at awaited
        values and renumber waits by rank among kept updates.
        """
        m = _json.loads(bir_json)
        changed = False
        for fn in m.get("functions", []):
            upd_order = {}     # sem id -> [instruction update dicts in order]
            upd_owner = {}     # sem id -> set of engines
            bad = set()        # sems we must not touch
            waits = {}         # sem id -> set of awaited values
            for blk in fn.get("blocks", []):
                for i in blk.get("instructions", []) or []:
                    si = i.get("sync_info")
                    if not si:
                        continue
                    dma = any(s in i.get("opcode", "") for s in _DMAISH)
                    for u in si.get("on_update") or []:
                        sid = u.get("id")
                        if (u.get("sync_type") != "semaphore"
                                or u.get("update_mode") != "sem-inc"
                                or u.get("update_value") != 1 or dma):
                            bad.add(sid)
                        upd_order.setdefault(sid, []).append(u)
                        upd_owner.setdefault(sid, set()).add(i.get("engine"))
                    for w in si.get("on_wait") or []:
                        sid = w.get("id")
                        if (w.get("sync_type") != "semaphore"
                                or w.get("wait_mode") != "sem-ge-imm"):
                            bad.add(sid)
                        else:
                            waits.setdefault(sid, set()).add(w.get("wait_value"))
            for sid, owners in upd_owner.items():
                if len(owners) != 1:
                    bad.add(sid)
            # decide kept values per sem
            keep = {}
            for sid, ups in upd_order.items():
                if sid in bad:
                    continue
                awaited = sorted(v for v in waits.get(sid, set())
                                 if v is not None and v > 0)
                total = len(ups)
                if awaited and awaited[-1] > total:
                    continue  # unexpected; leave untouched
                keep[sid] = set(awaited)
            if not keep:
                continue
            # rewrite updates (pass 2)
            counters = {sid: 0 for sid in keep}
            for blk in fn.get("blocks", []):
                for i in blk.get("instructions", []) or []:
                    si = i.get("sync_info")
                    if not si:
                        continue
                    ou = si.get("on_update") or []
                    if ou:
                        new = []
                        for u in ou:
                            sid = u.get("id")
                            if sid in keep:
                                counters[sid] += 1
                                if counters[sid] in keep[sid]:
                                    new.append(u)
                                else:
                                    changed = True
                            else:
                                new.append(u)
                        si["on_update"] = new
                    for w in si.get("on_wait") or []:
                        sid = w.get("id")
                        if sid in keep:
                            v = w.get("wait_value")
                            if v and v > 0:
                                kept_vals = keep[sid]
                                w["wait_value"] = sum(
                                    1 for kv in kept_vals if kv <= v)
        return _json.dumps(m).encode() if changed else bir_json

    def _patched_compile_bir(bir_json, tmpdir, neff_name="file.neff"):
        return _orig_compile_bir(
            _split_excess_waits(_sparsify_sems(bir_json)), tmpdir, neff_name)

    _bu.compile_bir_kernel = _patched_compile_bir
    _b2j.compile_bir_kernel = _patched_compile_bir
    _bu._ant_lstm_patched = True


# ---------------------------------------------------------------- program
def _build_program():
    from concourse import bass, tile
    import concourse.mybir as mybir

    dt = mybir.dt
    AF = mybir.ActivationFunctionType
    ALU = mybir.AluOpType

    nc = bass.Bass()

    # ---- kernel I/O (per-core shards, host-prepared layouts)
    xt = nc.declare_dram_parameter("xt", [D, NROWS], dt.bfloat16, isOutput=False)
    w2a = nc.declare_dram_parameter("w2a", [D, KT, 128], dt.bfloat16, isOutput=False)
    w2b = nc.declare_dram_parameter("w2b", [D, KT, 128], dt.bfloat16, isOutput=False)
    bias_q = nc.declare_dram_parameter("bias_q", [128, KT], dt.float32, isOutput=False)
    wih_t = nc.declare_dram_parameter("wih_t", [MT, 128, KT, 128], dt.bfloat16, isOutput=False)
    bih_t = nc.declare_dram_parameter("bih_t", [128, MT], dt.float32, isOutput=False)
    bhh_t = nc.declare_dram_parameter("bhh_t", [128, MT], dt.float32, isOutput=False)
    whh_t = nc.declare_dram_parameter("whh_t", [128, HK, MT, 128], dt.float8e4, isOutput=False)
    h0t = nc.declare_dram_parameter("h0t", [128, HK, BL], dt.float32, isOutput=False)
    c0t = nc.declare_dram_parameter("c0t", [128, HK, BL], dt.float32, isOutput=False)
    h2h_tt = nc.declare_dram_parameter("h2h_t", [HK, 128, HK, 128], dt.bfloat16, isOutput=False)
    h2b_t = nc.declare_dram_parameter("h2b_t", [128, HK], dt.float32, isOutput=False)
    outw_t = nc.declare_dram_parameter("outw_t", [128, HK, O], dt.bfloat16, isOutput=False)
    outb_t = nc.declare_dram_parameter("outb_t", [1, O], dt.bfloat16, isOutput=False)
    ident_in = nc.declare_dram_parameter("ident", [128, 128], dt.bfloat16, isOutput=False)
    out_d = nc.declare_dram_parameter("out", [NROWS, O], dt.float32, isOutput=True)

    # internal scratch: pre-gates for every (t, b), step-sliceable
    gxt = nc.dram_tensor("gxt", [MT, 128, S, BL], dt.bfloat16)

    NCH1 = NROWS // 512            # 8 column chunks of 512 in phase 1

    with tile.TileContext(nc) as tc:
        cpool = tc.alloc_tile_pool(name="const", bufs=1)
        ident = cpool.tile([128, 128], dt.bfloat16)
        nc.sync.dma_start(ident[:], ident_in[:])
        biasq_sb = cpool.tile([128, KT], dt.float32)
        nc.sync.dma_start(biasq_sb[:], bias_q[:])
        bg_sb = cpool.tile([128, MT], dt.float32)   # b_ih + b_hh
        bih_sb = cpool.tile([128, MT], dt.float32)
        bhh_sb = cpool.tile([128, MT], dt.float32)
        nc.sync.dma_start(bih_sb[:], bih_t[:])
        nc.sync.dma_start(bhh_sb[:], bhh_t[:])
        nc.vector.tensor_add(bg_sb[:], bih_sb[:], bhh_sb[:])
        h2b_sb = cpool.tile([128, HK], dt.float32)
        nc.sync.dma_start(h2b_sb[:], h2b_t[:])
        outw_sb = cpool.tile([128, HK, O], dt.bfloat16)
        nc.sync.dma_start(outw_sb[:], outw_t[:])
        outb_sb = cpool.tile([1, O], dt.bfloat16)
        nc.sync.dma_start(outb_sb[:], outb_t[:])
        ones_sb = cpool.tile([1, 128], dt.bfloat16)
        nc.vector.memset(ones_sb[:], 1.0)

        # ---------------- phase 1: conv + maxpool + pre-gates -> gxt
        with (
            tc.tile_pool(name="xtp", bufs=1) as xtp,
            tc.tile_pool(name="featp", bufs=1) as featp,
        ):
            xt_sb = xtp.tile([D, NROWS], dt.bfloat16)
            nc.sync.dma_start(xt_sb[:], xt[:])
            feat = featp.tile([128, KT, NROWS], dt.bfloat16)

            with nc.named_scope("p1_conv"):
                with (
                    tc.tile_pool(name="w2p", bufs=1) as w2p,
                    tc.tile_pool(name="psc", bufs=2, space="PSUM") as pscp,
                    tc.tile_pool(name="mx1", bufs=4) as mx1p,
                ):
                    w2a_sb = w2p.tile([D, KT, 128], dt.bfloat16)
                    w2b_sb = w2p.tile([D, KT, 128], dt.bfloat16)
                    nc.sync.dma_start(w2a_sb[:], w2a[:])
                    nc.sync.dma_start(w2b_sb[:], w2b[:])
                    for nch in range(NCH1):
                        cs = slice(nch * 512, (nch + 1) * 512)
                        for m in range(KT):
                            pa = pscp.tile([128, 512], dt.float32, tag="psA")
                            pb = pscp.tile([128, 512], dt.float32, tag="psB")
                            nc.tensor.matmul(pa[:], w2a_sb[:, m, :], xt_sb[:, cs],
                                             start=True, stop=True)
                            nc.tensor.matmul(pb[:], w2b_sb[:, m, :], xt_sb[:, cs],
                                             start=True, stop=True)
                            pbs = mx1p.tile([128, 512], dt.float32, tag="pbs")
                            nc.scalar.activation(pbs[:], pb[:], AF.Identity)
                            mx = mx1p.tile([128, 512], dt.float32)
                            nc.vector.tensor_max(mx[:], pa[:], pbs[:])
                            nc.scalar.activation(feat[:, m, cs], mx[:], AF.Relu,
                                                 bias=biasq_sb[:, m:m + 1])

            # pre-gates: gxt[m][p, t, b] = (w_ih @ feat)[gate row, (t, b)] + bias
            with nc.named_scope("p1_pregate"):
                with (
                    tc.tile_pool(name="wihp", bufs=3) as wihp,
                    tc.tile_pool(name="ps1", bufs=8, space="PSUM") as ps1p,
                    tc.tile_pool(name="gst", bufs=2) as gstp,
                ):
                    for m in range(MT):
                        wt = wihp.tile([128, KT, 128], dt.bfloat16)
                        nc.sync.dma_start(wt[:], wih_t[m])
                        pss = [ps1p.tile([128, 64, BL], dt.float32, tag="psG",
                                         name="psg%d" % n)
                               for n in range(NCH1)]
                        for k in range(KT):
                            for nch in range(NCH1):
                                cs = slice(nch * 512, (nch + 1) * 512)
                                nc.tensor.matmul(pss[nch][:], wt[:, k, :], feat[:, k, cs],
                                                 start=(k == 0), stop=(k == KT - 1))
                        gs = gstp.tile([128, NCH1, 64, BL], dt.bfloat16)
                        for nch in range(NCH1):
                            nc.scalar.activation(gs[:, nch, :, :], pss[nch][:], AF.Identity,
                                                 bias=bg_sb[:, m:m + 1])
                        nc.sync.dma_start(gxt[m], gs[:])

        # ---------------- phase 2: LSTM recurrence
        if PHASES < 2:
            cpool.release()
            return nc
        arch_pool = tc.alloc_tile_pool(name="arch", bufs=1)
        arch = arch_pool.tile([128, HK, S, BL], dt.bfloat16)

        with nc.named_scope("p2_lstm"):
            with (
                tc.tile_pool(name="whhp", bufs=1) as whhp,
                tc.tile_pool(name="state", bufs=4) as stp,
                tc.tile_pool(name="gxp", bufs=3) as gxp,
                tc.tile_pool(name="ps2", bufs=3, space="PSUM") as ps2p,
            ):
                whh_sb = whhp.tile([128, HK, MT, 128], dt.float8e4)
                nc.sync.dma_start(whh_sb[:], whh_t[:])

                hamf_mv = whhp.tile([128, 512], dt.bfloat16)
                nc.vector.memset(hamf_mv[:], 0.0)
                hamf_ps = None
                if HAM_FEED:
                    hamf_pool = tc.alloc_tile_pool(name="hamf", bufs=1,
                                                   space="PSUM")
                    hamf_ps = hamf_pool.tile([128, 512], dt.float32)

                def _ham_feed():
                    # dep-free full-width matmul; result never read
                    nc.tensor.matmul(hamf_ps[:], ident[:], hamf_mv[:],
                                     start=True, stop=True,
                                     skip_group_check=True)

                h0_sb = stp.tile([128, HK, BL], dt.float32, tag="h0")
                c0_sb = stp.tile([128, HK, BL], dt.float32, tag="c0")
                nc.sync.dma_start(h0_sb[:], h0t[:])
                nc.sync.dma_start(c0_sb[:], c0t[:])
                hA = stp.tile([128, 4, BL], dt.bfloat16, tag="hA")
                hB = stp.tile([128, 4, BL], dt.bfloat16, tag="hB")
                cA = stp.tile([128, 4, BL], dt.float32, tag="cA")
                cB = stp.tile([128, 4, BL], dt.float32, tag="cB")
                nc.vector.tensor_copy(hA[:], h0_sb[:, 0:4, :])
                nc.vector.tensor_copy(hB[:], h0_sb[:, 4:8, :])
                nc.vector.tensor_copy(cA[:], c0_sb[:, 0:4, :])
                nc.vector.tensor_copy(cB[:], c0_sb[:, 4:8, :])

                GXB = 32
                NBLK = (N_STEPS + GXB - 1) // GXB
                gxblks = {}

                def _prefetch(blk):
                    if blk >= NBLK:
                        return
                    gb = gxp.tile([128, MT, GXB, BL], dt.bfloat16, tag="gx")
                    t0 = blk * GXB
                    for m in range(MT):
                        nc.sync.dma_start(gb[:, m, :, :], gxt[m][:, t0:t0 + GXB, :])
                    gxblks[blk] = gb

                _prefetch(0)
                _prefetch(1)

                DS = 1.0 / WHH_SCALE

                def _chain(ps, gx_ap, c_prev, lo, hi, t, htag, ctag):
                    # ps[:, 0] holds the k0..3 partial, ps[:, 1] the k4..7
                    # partial (separate accumulation groups so each group is
                    # contiguous within its psum zero region).
                    u = stp.tile([128, 16, BL], dt.float32, tag="u" + htag)
                    nc.vector.scalar_tensor_tensor(
                        u[:], ps[:, 0, :, :], DS, gx_ap, ALU.mult, ALU.add)
                    gsum = stp.tile([128, 16, BL], dt.float32, tag="gs" + htag)
                    nc.vector.scalar_tensor_tensor(
                        gsum[:], ps[:, 1, :, :], DS, u[:], ALU.mult, ALU.add)
                    sig = stp.tile([128, 12, BL], dt.float32, tag="sg" + htag)
                    nc.scalar.activation(sig[:], gsum[:, 0:12, :], AF.Sigmoid)
                    gg = stp.tile([128, 4, BL], dt.float32, tag="gg" + htag)
                    nc.scalar.activation(gg[:], gsum[:, 12:16, :], AF.Tanh)
                    t1 = stp.tile([128, 4, BL], dt.float32, tag="t1" + htag)
                    nc.vector.tensor_mul(t1[:], sig[:, 0:4, :], gg[:])
                    t2 = stp.tile([128, 4, BL], dt.float32, tag="t2" + htag)
                    nc.gpsimd.tensor_mul(t2[:], sig[:, 4:8, :], c_prev[:])
                    c_new = stp.tile([128, 4, BL], dt.float32, tag=ctag)
                    nc.vector.tensor_add(c_new[:], t1[:], t2[:])
                    tc_ = stp.tile([128, 4, BL], dt.float32, tag="tc" + htag)
                    nc.scalar.activation(tc_[:], c_new[:], AF.Tanh)
                    h_new = stp.tile([128, 4, BL], dt.bfloat16, tag=htag)
                    nc.vector.tensor_mul(h_new[:], sig[:, 8:12, :], tc_[:])
                    nc.scalar.activation(arch[:, lo:hi, t, :], h_new[:], AF.Relu)
                    return h_new, c_new

                for t in range(N_STEPS):
                    blk, off = divmod(t, GXB)
                    if off == 0 and t > 0:
                        _prefetch(blk + 1)
                        gxblks.pop(blk - 1, None)
                    gxb = gxblks[blk]
                    ps_A = ps2p.tile([128, 2, 16, BL], dt.float32, tag="psA")
                    ps_B = ps2p.tile([128, 2, 16, BL], dt.float32, tag="psB")
                    # k 0..3 for both halves (needs only hA from t-1); each
                    # mi's group is contiguous: start at k0, stop at k3.
                    for half, ps in ((0, ps_A), (1, ps_B)):
                        for mi in range(16):
                            m = half * 16 + mi
                            for k in range(4):
                                nc.tensor.matmul(ps[:, 0, mi, :],
                                                 whh_sb[:, k, m, :],
                                                 hA[:, k, :],
                                                 start=(k == 0), stop=(k == 3))
                    if HAM_FEED >= 2:
                        _ham_feed()
                    # A-half k 4..7 (needs hB from t-1)
                    for mi in range(16):
                        for k in range(4, 8):
                            nc.tensor.matmul(ps_A[:, 1, mi, :],
                                             whh_sb[:, k, mi, :],
                                             hB[:, k - 4, :],
                                             start=(k == 4), stop=(k == 7))
                    hA, cA = _chain(ps_A, gxb[:, 0:16, off, :], cA, 0, 4, t,
                                    "hA", "cA")
                    # B-half k 4..7
                    for mi in range(16):
                        m = 16 + mi
                        for k in range(4, 8):
                            nc.tensor.matmul(ps_B[:, 1, mi, :],
                                             whh_sb[:, k, m, :],
                                             hB[:, k - 4, :],
                                             start=(k == 4), stop=(k == 7))
                    if HAM_FEED >= 1:
                        _ham_feed()
                    hB, cB = _chain(ps_B, gxb[:, 16:32, off, :], cB, 4, 8, t,
                                    "hB", "cB")

                if hamf_ps is not None:
                    hamf_pool.release()

        # ---------------- phase 3: h2 = relu(hs @ h2h.T + b); logits; log_softmax
        if PHASES < 3:
            arch_pool.release()
            cpool.release()
            return nc
        with nc.named_scope("p3_head"):
            with tc.tile_pool(name="h2p", bufs=1) as h2p:
                NCH3 = N_STEPS * BL // 512 if N_STEPS * BL >= 512 else 1
                CW = min(512, N_STEPS * BL)
                h2_sb = h2p.tile([128, HK, NROWS], dt.bfloat16)
                with (
                    tc.tile_pool(name="h2hp", bufs=4) as h2hp,
                    tc.tile_pool(name="ps3", bufs=8, space="PSUM") as ps3p,
                ):
                    for m in range(HK):
                        wt = h2hp.tile([128, HK, 128], dt.bfloat16)
                        nc.sync.dma_start(wt[:], h2h_tt[m])
                        pss = [ps3p.tile([128, CW // BL, BL], dt.float32, tag="psH",
                                         name="ps3_%d" % n)
                               for n in range(NCH3)]
                        for k in range(HK):
                            for nch in range(NCH3):
                                ts = slice(nch * (CW // BL), (nch + 1) * (CW // BL))
                                nc.tensor.matmul(pss[nch][:], wt[:, k, :], arch[:, k, ts, :],
                                                 start=(k == 0), stop=(k == HK - 1))
                        for nch in range(NCH3):
                            cs = slice(nch * CW, (nch + 1) * CW)
                            nc.scalar.activation(h2_sb[:, m, cs], pss[nch][:], AF.Relu,
                                                 bias=h2b_sb[:, m:m + 1])

                with (
                    tc.tile_pool(name="ps4", bufs=4, space="PSUM") as ps4p,
                    tc.tile_pool(name="lsp", bufs=4) as lsp,
                ):
                    NRC = (N_STEPS * BL) // 128
                    for rc in range(NRC):
                        p4 = ps4p.tile([128, O], dt.float32)
                        rs = slice(rc * 128, (rc + 1) * 128)
                        for k in range(HK):
                            nc.tensor.matmul(p4[:], h2_sb[:, k, rs], outw_sb[:, k, :],
                                             start=(k == 0), stop=False,
                                             skip_group_check=True)
                        nc.tensor.matmul(p4[:], ones_sb[:], outb_sb[:],
                                         start=False, stop=True, skip_group_check=True)
                        mx = lsp.tile([128, 1], dt.float32, tag="mx")
                        nc.vector.tensor_reduce(mx[:], p4[:], mybir.AxisListType.X,
                                                mybir.AluOpType.max, negate=True)
                        ex = lsp.tile([128, O], dt.float32, tag="ex")
                        se = lsp.tile([128, 1], dt.float32, tag="se")
                        nc.scalar.activation(ex[:], p4[:], AF.Exp,
                                             bias=mx[:, 0:1], accum_out=se[:])
                        lnse = lsp.tile([128, 1], dt.float32, tag="ln")
                        nc.scalar.activation(lnse[:], se[:], AF.Ln)
                        shift = lsp.tile([128, 1], dt.float32, tag="sh")
                        nc.vector.tensor_sub(shift[:], mx[:], lnse[:])  # -max - ln(sum)
                        outt = lsp.tile([128, O], dt.float32, tag="out")
                        nc.vector.tensor_scalar_add(outt[:], p4[:], shift[:, 0:1])
                        nc.sync.dma_start(out_d[rs, :], outt[:])

        arch_pool.release()
        cpool.release()

    return nc


# ---------------------------------------------------------------- host side
def _bf(x):
    return np.asarray(x, np.float32).astype(ml_dtypes.bfloat16)


def _f8(x, scale):
    return (np.asarray(x, np.float32) * scale).astype(ml_dtypes.float8_e4m3)


def _prep_core_inputs(inputs, r):
    """Build in_maps[r] — pure layout transforms of the full inputs."""
    bs = slice(r * BL, (r + 1) * BL)
    x = np.asarray(inputs["input_"], np.float32)[:, bs, :]       # [S, BL, D]
    xt = np.ascontiguousarray(x.transpose(2, 0, 1).reshape(D, NROWS))

    conv_w = np.asarray(inputs["conv_w"], np.float32)            # [OC,1,KW]
    conv_b = np.asarray(inputs["conv_b"], np.float32)
    w2a = np.zeros((D, KT, 128), np.float32)
    w2b = np.zeros((D, KT, 128), np.float32)
    bias_q = np.zeros((128, KT), np.float32)
    for m in range(KT):
        for mc in range(128):
            q = m * 128 + mc
            if q >= NF:
                continue
            c, j = q // AFTER_POOL, q % AFTER_POOL
            w2a[j:j + KW, m, mc] = conv_w[c, 0, :]
            if j + 1 + KW <= D:
                w2b[j + 1:j + 1 + KW, m, mc] = conv_w[c, 0, :]
            bias_q[mc, m] = conv_b[c]

    w_ih = np.asarray(inputs["w_ih"], np.float32)                # [G4, NF]
    w_ih_p = np.zeros((G4, NFP), np.float32)
    w_ih_p[:, :NF] = w_ih
    wih_t = np.zeros((MT, 128, KT, 128), np.float32)
    rows_of = [_gate_rows(m) for m in range(MT)]
    for m in range(MT):
        blk = w_ih_p[rows_of[m], :]                              # [128, NFP]
        for k in range(KT):
            wih_t[m, :, k, :] = blk[:, k * 128:(k + 1) * 128].T
    w_hh = np.asarray(inputs["w_hh"], np.float32)                # [G4, H]
    whh_t = np.zeros((128, HK, MT, 128), np.float32)
    for m in range(MT):
        blk = w_hh[rows_of[m], :]
        for k in range(HK):
            whh_t[:, k, m, :] = blk[:, k * 128:(k + 1) * 128].T

    def _gvec(v):
        v = np.asarray(v, np.float32)
        out = np.zeros((128, MT), np.float32)
        for m in range(MT):
            out[:, m] = v[rows_of[m]]
        return out

    h2h_w = np.asarray(inputs["h2h_w"], np.float32)              # [H, H]
    h2h_t = np.zeros((HK, 128, HK, 128), np.float32)
    for m in range(HK):
        for k in range(HK):
            h2h_t[m, :, k, :] = h2h_w[m * 128:(m + 1) * 128, k * 128:(k + 1) * 128].T
    h2b = np.asarray(inputs["h2h_b"], np.float32).reshape(HK, 128).T.copy()

    out_w = np.asarray(inputs["out_w"], np.float32)              # [O, H]
    outw_t = np.ascontiguousarray(
        out_w.T.reshape(HK, 128, O).transpose(1, 0, 2))          # [128, HK, O]

    def _state_t(v):
        v = np.asarray(v, np.float32)[0, bs, :]                  # [BL, H]
        return np.ascontiguousarray(
            v.T.reshape(HK, 128, BL).transpose(1, 0, 2))         # [128, HK, BL]

    return {
        "xt": _bf(xt),
        "w2a": _bf(w2a), "w2b": _bf(w2b), "bias_q": bias_q,
        "wih_t": _bf(wih_t),
        "bih_t": _gvec(inputs["b_ih"]), "bhh_t": _gvec(inputs["b_hh"]),
        "whh_t": _f8(whh_t, WHH_SCALE),
        "h0t": _state_t(inputs["hidden"]), "c0t": _state_t(inputs["cell"]),
        "h2h_t": _bf(h2h_t), "h2b_t": h2b,
        "outw_t": _bf(outw_t), "outb_t": _bf(np.asarray(inputs["out_b"],
                                                        np.float32)[None, :]),
        "ident": _bf(np.eye(128, dtype=np.float32)),
    }


_CACHE = {}


def kernel(**inputs) -> np.ndarray:
    _install_patches()
    from concourse.bass_utils import run_bass_kernel_spmd

    if "nc" not in _CACHE:
        _CACHE["nc"] = _build_program()
    nc = _CACHE["nc"]

    in_maps = [_prep_core_inputs(inputs, r) for r in range(N_CORES)]
    res = run_bass_kernel_spmd(nc, in_maps, list(range(N_CORES)),
                               trace=bool(os.environ.get("BASS_TRACE_RUN")))
    _CACHE["last_result"] = res

    out = np.zeros((S, B, O), np.float32)
    for r in range(N_CORES):
        o = res.results[r]["out"].reshape(S, BL, O)
        out[:, r * BL:(r + 1) * BL, :] = o
    return out


# revision 23
# speedup vs baseline: 1.3721x; 1.0815x over previous
"""CNN+LSTM recognizer on 8 Trainium2 NeuronCores.

Data-parallel over the batch axis (8 samples per core, zero cross-core
communication). All weights are replicated; each core runs conv -> maxpool
-> pre-gate matmul (the x @ w_ih.T part of every LSTM step, batched over
time) -> the 512-step recurrence -> MLP head -> log_softmax on its batch
shard.

Recurrence design notes:
- w_hh is stored as fp8 e4m3 (x256 scale) so LDWEIGHTS runs at the 4x
  fast-weight-load rate; the descale folds into the existing
  scalar_tensor_tensor that adds the precomputed input gates.
- Gate tiles are grouped into two hidden-chunk halves (A = chunks 0..3,
  B = 4..7). Each step issues matmuls as [A|B @ k0..3], [A @ k4..7],
  chainA, [B @ k4..7], chainB, so each half's sigmoid/tanh/cell-update
  chain executes while the tensor engine streams the other half's
  matmuls, and the next step's k0..3 matmuls only need chainA's output.
"""

import os
import sys

sys.path.insert(0, "/opt/trn_rl_repo")

import json as _json

import ml_dtypes
import numpy as np

# ---------------------------------------------------------------- constants
S, B, D = 512, 64, 120
OC, KW = 16, 6
AFTER_CONV = (D - KW) + 1          # 115
AFTER_POOL = AFTER_CONV - 1        # 114
NF = OC * AFTER_POOL               # 1824 LSTM input features
NFP = 1920                         # padded to 15 * 128
H, O = 1024, 48
G4 = 4 * H                         # 4096 gate rows
N_CORES = 8
BL = B // N_CORES                  # 8 samples per core
NROWS = S * BL                     # 4096 (s, b) rows per core
KT = NFP // 128                    # 15 k-tiles for pre-gates
KT2 = 16                           # padded to even for fp8 DoubleRow pairs
F8S = 16.0                         # fp8 scale for feat / arch activations
W8S = 256.0                        # fp8 scale for w_ih / h2h weights
MT = G4 // 128                     # 32 gate m-tiles
HK = H // 128                      # 8 hidden chunks
N_STEPS = int(os.environ.get("BASS_LSTM_STEPS", str(S)))
PHASES = int(os.environ.get("BASS_PHASES", "3"))
WHH_SCALE = 256.0
# N=512 dummy matmuls issued per LSTM step to keep the PE HAM clock-gate
# released (the real FD=8 matmuls alone leave the PE array duty cycle so low
# that HAM throttles the PE clock to 1.2 GHz for the whole recurrence).
HAM_FEED = int(os.environ.get("BASS_HAM_FEED", "0"))

# gate-tile order: two halves of hidden chunks, gate-type major inside each:
#   m' in [0,16):  half A (chunks 0..3),  m' = gt*4 + c        (c in 0..3)
#   m' in [16,32): half B (chunks 4..7),  m' = 16 + gt*4 + (c-4)
# gt: 0=i, 1=f, 2=o, 3=g (torch rows i,f,g,o -> bases 0,H,3H,2H)
_GATE_BASE = [0, H, 3 * H, 2 * H]


def _gate_rows(m):
    if m < 16:
        gt, c = m // 4, m % 4
    else:
        gt, c = (m - 16) // 4, 4 + (m - 16) % 4
    base = _GATE_BASE[gt] + c * 128
    return np.arange(base, base + 128)


# ---------------------------------------------------------------- harness patches
def _install_patches():
    from concourse import tile
    import concourse.mybir as mybir
    import concourse.bass_utils as _bu
    import concourse.bass2jax as _b2j
    from concourse.vector_clock import ScopedClock

    if getattr(_bu, "_ant_lstm_patched", False):
        return

    def _patched_dab(self, tick_clock, wait_clock):
        # This walrus rejects >2 sem waits on one instruction; the tile tail
        # drain waits on every ticked proc. Spread waits over nop carriers.
        nc = self.nc
        carrier = nc.sync.nop(nofuse=True)
        wait_clock.add_sem_waits(
            carrier.ins, ScopedClock({None: tick_clock.global_clock})
        )
        si = carrier.ins.sync_info
        if si is not None and si.on_wait and len(si.on_wait) > 1:
            waits = list(si.on_wait)
            si.on_wait = waits[:1]
            for w in waits[1:]:
                extra = nc.sync.nop(nofuse=True)
                extra.ins.sync_info = mybir.SyncInfo(on_wait=[w], on_update=[])
        nc.sync.drain()
        nc.all_engine_barrier()
        popped = nc._tile_sem_poison_stack.pop()
        assert popped is self._sem_poison
        nc.clear_and_free_semaphores(list(self.sems.allocated().values()))
        nc.all_engine_barrier()

    tile.TileContext._drain_and_barrier = _patched_dab

    _MAXW = 1
    _orig_compile_bir = _bu.compile_bir_kernel

    def _split_excess_waits(bir_json: bytes) -> bytes:
        m = _json.loads(bir_json)
        changed = False
        for fn in m.get("functions", []):
            for blk in fn.get("blocks", []):
                insts = blk.get("instructions")
                if not insts:
                    continue
                out = []
                for i in insts:
                    si = i.get("sync_info")
                    ow = (si or {}).get("on_wait") or []
                    if len(ow) > _MAXW:
                        changed = True
                        extra, keep = ow[:-_MAXW], ow[-_MAXW:]
                        for k in range(0, len(extra), _MAXW):
                            out.append({
                                "debug": i.get("debug", 0),
                                "engine": i["engine"],
                                "ins": [], "outs": [],
                                "name": i["name"] + "_w%d" % k,
                                "opcode": "NoOp",
                                "sync_info": {"on_update": [],
                                              "on_wait": extra[k:k + _MAXW]},
                            })
                        si["on_wait"] = keep
                    out.append(i)
                blk["instructions"] = out
        return _json.dumps(m).encode() if changed else bir_json

    _DMAISH = ("DMA", "Trigger", "Collective")

    def _sparsify_sems(bir_json: bytes) -> bytes:
        """Drop per-instruction sem-inc updates nobody waits on.

        Tile ticks a per-engine semaphore on every instruction; the EVT_SEM
        register writes serialize at ~26 ns each, which caps the tensor
        engine at ~34 ns per matmul in the LSTM inner loop. Engine streams
        complete in program order, so a wait for "count >= v" is equivalent
        to a wait on the v-th updater alone. Keep an update only at awaited
        values and renumber waits by rank among kept updates.
        """
        m = _json.loads(bir_json)
        changed = False
        for fn in m.get("functions", []):
            upd_order = {}     # sem id -> [instruction update dicts in order]
            upd_owner = {}     # sem id -> set of engines
            bad = set()        # sems we must not touch
            waits = {}         # sem id -> set of awaited values
            for blk in fn.get("blocks", []):
                for i in blk.get("instructions", []) or []:
                    si = i.get("sync_info")
                    if not si:
                        continue
                    dma = any(s in i.get("opcode", "") for s in _DMAISH)
                    for u in si.get("on_update") or []:
                        sid = u.get("id")
                        if (u.get("sync_type") != "semaphore"
                                or u.get("update_mode") != "sem-inc"
                                or u.get("update_value") != 1 or dma):
                            bad.add(sid)
                        upd_order.setdefault(sid, []).append(u)
                        upd_owner.setdefault(sid, set()).add(i.get("engine"))
                    for w in si.get("on_wait") or []:
                        sid = w.get("id")
                        if (w.get("sync_type") != "semaphore"
                                or w.get("wait_mode") != "sem-ge-imm"):
                            bad.add(sid)
                        else:
                            waits.setdefault(sid, set()).add(w.get("wait_value"))
            for sid, owners in upd_owner.items():
                if len(owners) != 1:
                    bad.add(sid)
            # decide kept values per sem
            keep = {}
            for sid, ups in upd_order.items():
                if sid in bad:
                    continue
                awaited = sorted(v for v in waits.get(sid, set())
                                 if v is not None and v > 0)
                total = len(ups)
                if awaited and awaited[-1] > total:
                    continue  # unexpected; leave untouched
                keep[sid] = set(awaited)
            if not keep:
                continue
            # rewrite updates (pass 2)
            counters = {sid: 0 for sid in keep}
            for blk in fn.get("blocks", []):
                for i in blk.get("instructions", []) or []:
                    si = i.get("sync_info")
                    if not si:
                        continue
                    ou = si.get("on_update") or []
                    if ou:
                        new = []
                        for u in ou:
                            sid = u.get("id")
                            if sid in keep:
                                counters[sid] += 1
                                if counters[sid] in keep[sid]:
                                    new.append(u)
                                else:
                                    changed = True
                            else:
                                new.append(u)
                        si["on_update"] = new
                    for w in si.get("on_wait") or []:
                        sid = w.get("id")
                        if sid in keep:
                            v = w.get("wait_value")
                            if v and v > 0:
                                kept_vals = keep[sid]
                                w["wait_value"] = sum(
                                    1 for kv in kept_vals if kv <= v)
        return _json.dumps(m).encode() if changed else bir_json

    def _patched_compile_bir(bir_json, tmpdir, neff_name="file.neff"):
        return _orig_compile_bir(
            _split_excess_waits(_sparsify_sems(bir_json)), tmpdir, neff_name)

    _bu.compile_bir_kernel = _patched_compile_bir
    _b2j.compile_bir_kernel = _patched_compile_bir
    _bu._ant_lstm_patched = True


# ---------------------------------------------------------------- program
def _build_program():
    from concourse import bass, tile
    import concourse.mybir as mybir

    dt = mybir.dt
    AF = mybir.ActivationFunctionType
    ALU = mybir.AluOpType

    nc = bass.Bass()

    # ---- kernel I/O (per-core shards, host-prepared layouts)
    xt = nc.declare_dram_parameter("xt", [D, NROWS], dt.bfloat16, isOutput=False)
    w2a = nc.declare_dram_parameter("w2a", [D, KT, 128], dt.bfloat16, isOutput=False)
    w2b = nc.declare_dram_parameter("w2b", [D, KT, 128], dt.bfloat16, isOutput=False)
    bias_q = nc.declare_dram_parameter("bias_q", [128, KT], dt.float32, isOutput=False)
    wih_t = nc.declare_dram_parameter("wih_t", [MT, 128, KT2, 128], dt.float8e4, isOutput=False)
    bih_t = nc.declare_dram_parameter("bih_t", [128, MT], dt.float32, isOutput=False)
    bhh_t = nc.declare_dram_parameter("bhh_t", [128, MT], dt.float32, isOutput=False)
    whh_t = nc.declare_dram_parameter("whh_t", [128, HK, MT, 128], dt.float8e4, isOutput=False)
    h0t = nc.declare_dram_parameter("h0t", [128, HK, BL], dt.float32, isOutput=False)
    c0t = nc.declare_dram_parameter("c0t", [128, HK, BL], dt.float32, isOutput=False)
    h2h_tt = nc.declare_dram_parameter("h2h_t", [HK, 128, HK, 128], dt.float8e4, isOutput=False)
    h2b_t = nc.declare_dram_parameter("h2b_t", [128, HK], dt.float32, isOutput=False)
    outw_t = nc.declare_dram_parameter("outw_t", [128, HK, O], dt.bfloat16, isOutput=False)
    outb_t = nc.declare_dram_parameter("outb_t", [1, O], dt.bfloat16, isOutput=False)
    ident_in = nc.declare_dram_parameter("ident", [128, 128], dt.bfloat16, isOutput=False)
    out_d = nc.declare_dram_parameter("out", [NROWS, O], dt.float32, isOutput=True)

    # internal scratch: pre-gates for every (t, b), step-sliceable
    gxt = nc.dram_tensor("gxt", [MT, 128, S, BL], dt.bfloat16)

    NCH1 = NROWS // 512            # 8 column chunks of 512 in phase 1

    with tile.TileContext(nc) as tc:
        cpool = tc.alloc_tile_pool(name="const", bufs=1)
        ident = cpool.tile([128, 128], dt.bfloat16)
        nc.sync.dma_start(ident[:], ident_in[:])
        biasq_sb = cpool.tile([128, KT], dt.float32)
        nc.sync.dma_start(biasq_sb[:], bias_q[:])
        bg_sb = cpool.tile([128, MT], dt.float32)   # b_ih + b_hh
        bih_sb = cpool.tile([128, MT], dt.float32)
        bhh_sb = cpool.tile([128, MT], dt.float32)
        nc.sync.dma_start(bih_sb[:], bih_t[:])
        nc.sync.dma_start(bhh_sb[:], bhh_t[:])
        nc.vector.tensor_add(bg_sb[:], bih_sb[:], bhh_sb[:])
        h2b_sb = cpool.tile([128, HK], dt.float32)
        nc.sync.dma_start(h2b_sb[:], h2b_t[:])
        outw_sb = cpool.tile([128, HK, O], dt.bfloat16)
        nc.sync.dma_start(outw_sb[:], outw_t[:])
        outb_sb = cpool.tile([1, O], dt.bfloat16)
        nc.sync.dma_start(outb_sb[:], outb_t[:])
        ones_sb = cpool.tile([1, 128], dt.bfloat16)
        nc.vector.memset(ones_sb[:], 1.0)

        # ---------------- phase 1: conv + maxpool + pre-gates -> gxt
        with (
            tc.tile_pool(name="xtp", bufs=1) as xtp,
            tc.tile_pool(name="featp", bufs=1) as featp,
        ):
            xt_sb = xtp.tile([D, NROWS], dt.bfloat16)
            nc.sync.dma_start(xt_sb[:], xt[:])
            feat = featp.tile([128, KT2, NROWS], dt.float8e4)
            nc.vector.memset(feat[:, KT, :], 0.0)   # zero pad tile 15

            with nc.named_scope("p1_conv"):
                with (
                    tc.tile_pool(name="w2p", bufs=1) as w2p,
                    tc.tile_pool(name="psc", bufs=2, space="PSUM") as pscp,
                    tc.tile_pool(name="mx1", bufs=4) as mx1p,
                ):
                    w2a_sb = w2p.tile([D, KT, 128], dt.bfloat16)
                    w2b_sb = w2p.tile([D, KT, 128], dt.bfloat16)
                    nc.sync.dma_start(w2a_sb[:], w2a[:])
                    nc.sync.dma_start(w2b_sb[:], w2b[:])
                    for nch in range(NCH1):
                        cs = slice(nch * 512, (nch + 1) * 512)
                        for m in range(KT):
                            pa = pscp.tile([128, 512], dt.float32, tag="psA")
                            pb = pscp.tile([128, 512], dt.float32, tag="psB")
                            nc.tensor.matmul(pa[:], w2a_sb[:, m, :], xt_sb[:, cs],
                                             start=True, stop=True)
                            nc.tensor.matmul(pb[:], w2b_sb[:, m, :], xt_sb[:, cs],
                                             start=True, stop=True)
                            pbs = mx1p.tile([128, 512], dt.float32, tag="pbs")
                            nc.scalar.activation(pbs[:], pb[:], AF.Identity)
                            mx = mx1p.tile([128, 512], dt.float32)
                            nc.vector.tensor_max(mx[:], pa[:], pbs[:])
                            # bias_q is pre-scaled by F8S on the host, so this
                            # writes F8S * relu(conv + bias) into fp8 feat
                            nc.scalar.activation(feat[:, m, cs], mx[:], AF.Relu,
                                                 bias=biasq_sb[:, m:m + 1],
                                                 scale=F8S)

            # pre-gates: gxt[m][p, t, b] = (w_ih @ feat)[gate row, (t, b)] + bias
            with nc.named_scope("p1_pregate"):
                with (
                    tc.tile_pool(name="wihp", bufs=3) as wihp,
                    tc.tile_pool(name="ps1", bufs=8, space="PSUM") as ps1p,
                    tc.tile_pool(name="gst", bufs=2) as gstp,
                ):
                    for m in range(MT):
                        wt = wihp.tile([128, KT2, 128], dt.float8e4)
                        nc.sync.dma_start(wt[:], wih_t[m])
                        pss = [ps1p.tile([128, 64, BL], dt.float32, tag="psG",
                                         name="psg%d" % n)
                               for n in range(NCH1)]
                        for kk in range(KT2 // 2):
                            for nch in range(NCH1):
                                cs = slice(nch * 512, (nch + 1) * 512)
                                nc.tensor.matmul(
                                    pss[nch][:], wt[:, 2 * kk:2 * kk + 2, :],
                                    feat[:, 2 * kk:2 * kk + 2, cs],
                                    start=(kk == 0), stop=(kk == KT2 // 2 - 1),
                                    perf_mode=mybir.MatmulPerfMode.DoubleRow)
                        gs = gstp.tile([128, NCH1, 64, BL], dt.bfloat16)
                        for nch in range(NCH1):
                            nc.scalar.activation(gs[:, nch, :, :], pss[nch][:], AF.Identity,
                                                 bias=bg_sb[:, m:m + 1],
                                                 scale=1.0 / (F8S * W8S))
                        nc.sync.dma_start(gxt[m], gs[:])

        # ---------------- phase 2: LSTM recurrence
        if PHASES < 2:
            cpool.release()
            return nc
        arch_pool = tc.alloc_tile_pool(name="arch", bufs=1)
        arch = arch_pool.tile([128, HK, S, BL], dt.float8e4)

        with nc.named_scope("p2_lstm"):
            with (
                tc.tile_pool(name="whhp", bufs=1) as whhp,
                tc.tile_pool(name="state", bufs=4) as stp,
                tc.tile_pool(name="gxp", bufs=3) as gxp,
                tc.tile_pool(name="ps2", bufs=3, space="PSUM") as ps2p,
            ):
                whh_sb = whhp.tile([128, HK, MT, 128], dt.float8e4)
                nc.sync.dma_start(whh_sb[:], whh_t[:])

                hamf_mv = whhp.tile([128, 512], dt.bfloat16)
                nc.vector.memset(hamf_mv[:], 0.0)
                hamf_ps = None
                if HAM_FEED:
                    hamf_pool = tc.alloc_tile_pool(name="hamf", bufs=1,
                                                   space="PSUM")
                    hamf_ps = hamf_pool.tile([128, 512], dt.float32)

                def _ham_feed():
                    # dep-free full-width matmul; result never read
                    nc.tensor.matmul(hamf_ps[:], ident[:], hamf_mv[:],
                                     start=True, stop=True,
                                     skip_group_check=True)

                h0_sb = stp.tile([128, HK, BL], dt.float32, tag="h0")
                c0_sb = stp.tile([128, HK, BL], dt.float32, tag="c0")
                nc.sync.dma_start(h0_sb[:], h0t[:])
                nc.sync.dma_start(c0_sb[:], c0t[:])
                hA = stp.tile([128, 4, BL], dt.bfloat16, tag="hA")
                hB = stp.tile([128, 4, BL], dt.bfloat16, tag="hB")
                cA = stp.tile([128, 4, BL], dt.float32, tag="cA")
                cB = stp.tile([128, 4, BL], dt.float32, tag="cB")
                nc.vector.tensor_copy(hA[:], h0_sb[:, 0:4, :])
                nc.vector.tensor_copy(hB[:], h0_sb[:, 4:8, :])
                nc.vector.tensor_copy(cA[:], c0_sb[:, 0:4, :])
                nc.vector.tensor_copy(cB[:], c0_sb[:, 4:8, :])

                GXB = 32
                NBLK = (N_STEPS + GXB - 1) // GXB
                gxblks = {}

                def _prefetch(blk):
                    if blk >= NBLK:
                        return
                    gb = gxp.tile([128, MT, GXB, BL], dt.bfloat16, tag="gx")
                    t0 = blk * GXB
                    for m in range(MT):
                        nc.sync.dma_start(gb[:, m, :, :], gxt[m][:, t0:t0 + GXB, :])
                    gxblks[blk] = gb

                _prefetch(0)
                _prefetch(1)

                DS = 1.0 / WHH_SCALE

                def _chain(ps, gx_ap, c_prev, lo, hi, t, htag, ctag):
                    # ps[:, 0] holds the k0..3 partial, ps[:, 1] the k4..7
                    # partial (separate accumulation groups so each group is
                    # contiguous within its psum zero region).
                    u = stp.tile([128, 16, BL], dt.float32, tag="u" + htag)
                    nc.vector.scalar_tensor_tensor(
                        u[:], ps[:, 0, :, :], DS, gx_ap, ALU.mult, ALU.add)
                    gsum = stp.tile([128, 16, BL], dt.float32, tag="gs" + htag)
                    nc.vector.scalar_tensor_tensor(
                        gsum[:], ps[:, 1, :, :], DS, u[:], ALU.mult, ALU.add)
                    sig = stp.tile([128, 12, BL], dt.float32, tag="sg" + htag)
                    nc.scalar.activation(sig[:], gsum[:, 0:12, :], AF.Sigmoid)
                    gg = stp.tile([128, 4, BL], dt.float32, tag="gg" + htag)
                    nc.scalar.activation(gg[:], gsum[:, 12:16, :], AF.Tanh)
                    t1 = stp.tile([128, 4, BL], dt.float32, tag="t1" + htag)
                    nc.vector.tensor_mul(t1[:], sig[:, 0:4, :], gg[:])
                    t2 = stp.tile([128, 4, BL], dt.float32, tag="t2" + htag)
                    nc.gpsimd.tensor_mul(t2[:], sig[:, 4:8, :], c_prev[:])
                    c_new = stp.tile([128, 4, BL], dt.float32, tag=ctag)
                    nc.vector.tensor_add(c_new[:], t1[:], t2[:])
                    tc_ = stp.tile([128, 4, BL], dt.float32, tag="tc" + htag)
                    nc.scalar.activation(tc_[:], c_new[:], AF.Tanh)
                    h_new = stp.tile([128, 4, BL], dt.bfloat16, tag=htag)
                    nc.vector.tensor_mul(h_new[:], sig[:, 8:12, :], tc_[:])
                    nc.scalar.activation(arch[:, lo:hi, t, :], h_new[:], AF.Relu,
                                         scale=F8S)
                    return h_new, c_new

                for t in range(N_STEPS):
                    blk, off = divmod(t, GXB)
                    if off == 0 and t > 0:
                        _prefetch(blk + 1)
                        gxblks.pop(blk - 1, None)
                    gxb = gxblks[blk]
                    ps_A = ps2p.tile([128, 2, 16, BL], dt.float32, tag="psA")
                    ps_B = ps2p.tile([128, 2, 16, BL], dt.float32, tag="psB")
                    # k 0..3 for both halves (needs only hA from t-1); each
                    # mi's group is contiguous: start at k0, stop at k3.
                    for half, ps in ((0, ps_A), (1, ps_B)):
                        for mi in range(16):
                            m = half * 16 + mi
                            for k in range(4):
                                nc.tensor.matmul(ps[:, 0, mi, :],
                                                 whh_sb[:, k, m, :],
                                                 hA[:, k, :],
                                                 start=(k == 0), stop=(k == 3))
                    if HAM_FEED >= 2:
                        _ham_feed()
                    # A-half k 4..7 (needs hB from t-1)
                    for mi in range(16):
                        for k in range(4, 8):
                            nc.tensor.matmul(ps_A[:, 1, mi, :],
                                             whh_sb[:, k, mi, :],
                                             hB[:, k - 4, :],
                                             start=(k == 4), stop=(k == 7))
                    hA, cA = _chain(ps_A, gxb[:, 0:16, off, :], cA, 0, 4, t,
                                    "hA", "cA")
                    # B-half k 4..7
                    for mi in range(16):
                        m = 16 + mi
                        for k in range(4, 8):
                            nc.tensor.matmul(ps_B[:, 1, mi, :],
                                             whh_sb[:, k, m, :],
                                             hB[:, k - 4, :],
                                             start=(k == 4), stop=(k == 7))
                    if HAM_FEED >= 1:
                        _ham_feed()
                    hB, cB = _chain(ps_B, gxb[:, 16:32, off, :], cB, 4, 8, t,
                                    "hB", "cB")

                if hamf_ps is not None:
                    hamf_pool.release()

        # ---------------- phase 3: h2 = relu(hs @ h2h.T + b); logits; log_softmax
        if PHASES < 3:
            arch_pool.release()
            cpool.release()
            return nc
        with nc.named_scope("p3_head"):
            with tc.tile_pool(name="h2p", bufs=1) as h2p:
                NCH3 = N_STEPS * BL // 512 if N_STEPS * BL >= 512 else 1
                CW = min(512, N_STEPS * BL)
                h2_sb = h2p.tile([128, HK, NROWS], dt.bfloat16)
                with (
                    tc.tile_pool(name="h2hp", bufs=4) as h2hp,
                    tc.tile_pool(name="ps3", bufs=8, space="PSUM") as ps3p,
                ):
                    for m in range(HK):
                        wt = h2hp.tile([128, HK, 128], dt.float8e4)
                        nc.sync.dma_start(wt[:], h2h_tt[m])
                        pss = [ps3p.tile([128, CW // BL, BL], dt.float32, tag="psH",
                                         name="ps3_%d" % n)
                               for n in range(NCH3)]
                        for kk in range(HK // 2):
                            for nch in range(NCH3):
                                ts = slice(nch * (CW // BL), (nch + 1) * (CW // BL))
                                nc.tensor.matmul(
                                    pss[nch][:], wt[:, 2 * kk:2 * kk + 2, :],
                                    arch[:, 2 * kk:2 * kk + 2, ts, :],
                                    start=(kk == 0), stop=(kk == HK // 2 - 1),
                                    perf_mode=mybir.MatmulPerfMode.DoubleRow)
                        for nch in range(NCH3):
                            cs = slice(nch * CW, (nch + 1) * CW)
                            nc.scalar.activation(h2_sb[:, m, cs], pss[nch][:], AF.Relu,
                                                 bias=h2b_sb[:, m:m + 1],
                                                 scale=1.0 / (F8S * W8S))

                with (
                    tc.tile_pool(name="ps4", bufs=4, space="PSUM") as ps4p,
                    tc.tile_pool(name="lsp", bufs=4) as lsp,
                ):
                    NRC = (N_STEPS * BL) // 128
                    for rc in range(NRC):
                        p4 = ps4p.tile([128, O], dt.float32)
                        rs = slice(rc * 128, (rc + 1) * 128)
                        for k in range(HK):
                            nc.tensor.matmul(p4[:], h2_sb[:, k, rs], outw_sb[:, k, :],
                                             start=(k == 0), stop=False,
                                             skip_group_check=True)
                        nc.tensor.matmul(p4[:], ones_sb[:], outb_sb[:],
                                         start=False, stop=True, skip_group_check=True)
                        mx = lsp.tile([128, 1], dt.float32, tag="mx")
                        nc.vector.tensor_reduce(mx[:], p4[:], mybir.AxisListType.X,
                                                mybir.AluOpType.max, negate=True)
                        ex = lsp.tile([128, O], dt.float32, tag="ex")
                        se = lsp.tile([128, 1], dt.float32, tag="se")
                        nc.scalar.activation(ex[:], p4[:], AF.Exp,
                                             bias=mx[:, 0:1], accum_out=se[:])
                        lnse = lsp.tile([128, 1], dt.float32, tag="ln")
                        nc.scalar.activation(lnse[:], se[:], AF.Ln)
                        shift = lsp.tile([128, 1], dt.float32, tag="sh")
                        nc.vector.tensor_sub(shift[:], mx[:], lnse[:])  # -max - ln(sum)
                        outt = lsp.tile([128, O], dt.float32, tag="out")
                        nc.vector.tensor_scalar_add(outt[:], p4[:], shift[:, 0:1])
                        nc.sync.dma_start(out_d[rs, :], outt[:])

        arch_pool.release()
        cpool.release()

    return nc


# ---------------------------------------------------------------- host side
def _bf(x):
    return np.asarray(x, np.float32).astype(ml_dtypes.bfloat16)


def _f8(x, scale):
    return (np.asarray(x, np.float32) * scale).astype(ml_dtypes.float8_e4m3)


def _prep_core_inputs(inputs, r):
    """Build in_maps[r] — pure layout transforms of the full inputs."""
    bs = slice(r * BL, (r + 1) * BL)
    x = np.asarray(inputs["input_"], np.float32)[:, bs, :]       # [S, BL, D]
    xt = np.ascontiguousarray(x.transpose(2, 0, 1).reshape(D, NROWS))

    conv_w = np.asarray(inputs["conv_w"], np.float32)            # [OC,1,KW]
    conv_b = np.asarray(inputs["conv_b"], np.float32)
    w2a = np.zeros((D, KT, 128), np.float32)
    w2b = np.zeros((D, KT, 128), np.float32)
    bias_q = np.zeros((128, KT), np.float32)
    for m in range(KT):
        for mc in range(128):
            q = m * 128 + mc
            if q >= NF:
                continue
            c, j = q // AFTER_POOL, q % AFTER_POOL
            w2a[j:j + KW, m, mc] = conv_w[c, 0, :]
            if j + 1 + KW <= D:
                w2b[j + 1:j + 1 + KW, m, mc] = conv_w[c, 0, :]
            bias_q[mc, m] = conv_b[c] * F8S   # activation uses scale=F8S

    w_ih = np.asarray(inputs["w_ih"], np.float32)                # [G4, NF]
    w_ih_p = np.zeros((G4, KT2 * 128), np.float32)
    w_ih_p[:, :NF] = w_ih
    wih_t = np.zeros((MT, 128, KT2, 128), np.float32)
    rows_of = [_gate_rows(m) for m in range(MT)]
    for m in range(MT):
        blk = w_ih_p[rows_of[m], :]                              # [128, KT2*128]
        for k in range(KT2):
            wih_t[m, :, k, :] = blk[:, k * 128:(k + 1) * 128].T
    w_hh = np.asarray(inputs["w_hh"], np.float32)                # [G4, H]
    whh_t = np.zeros((128, HK, MT, 128), np.float32)
    for m in range(MT):
        blk = w_hh[rows_of[m], :]
        for k in range(HK):
            whh_t[:, k, m, :] = blk[:, k * 128:(k + 1) * 128].T

    def _gvec(v):
        v = np.asarray(v, np.float32)
        out = np.zeros((128, MT), np.float32)
        for m in range(MT):
            out[:, m] = v[rows_of[m]]
        return out

    h2h_w = np.asarray(inputs["h2h_w"], np.float32)              # [H, H]
    h2h_t = np.zeros((HK, 128, HK, 128), np.float32)
    for m in range(HK):
        for k in range(HK):
            h2h_t[m, :, k, :] = h2h_w[m * 128:(m + 1) * 128, k * 128:(k + 1) * 128].T
    h2b = np.asarray(inputs["h2h_b"], np.float32).reshape(HK, 128).T.copy()

    out_w = np.asarray(inputs["out_w"], np.float32)              # [O, H]
    outw_t = np.ascontiguousarray(
        out_w.T.reshape(HK, 128, O).transpose(1, 0, 2))          # [128, HK, O]

    def _state_t(v):
        v = np.asarray(v, np.float32)[0, bs, :]                  # [BL, H]
        return np.ascontiguousarray(
            v.T.reshape(HK, 128, BL).transpose(1, 0, 2))         # [128, HK, BL]

    return {
        "xt": _bf(xt),
        "w2a": _bf(w2a), "w2b": _bf(w2b), "bias_q": bias_q,
        "wih_t": _f8(wih_t, W8S),
        "bih_t": _gvec(inputs["b_ih"]), "bhh_t": _gvec(inputs["b_hh"]),
        "whh_t": _f8(whh_t, WHH_SCALE),
        "h0t": _state_t(inputs["hidden"]), "c0t": _state_t(inputs["cell"]),
        "h2h_t": _f8(h2h_t, W8S), "h2b_t": h2b,
        "outw_t": _bf(outw_t), "outb_t": _bf(np.asarray(inputs["out_b"],
                                                        np.float32)[None, :]),
        "ident": _bf(np.eye(128, dtype=np.float32)),
    }


_CACHE = {}


def kernel(**inputs) -> np.ndarray:
    _install_patches()
    from concourse.bass_utils import run_bass_kernel_spmd

    if "nc" not in _CACHE:
        _CACHE["nc"] = _build_program()
    nc = _CACHE["nc"]

    in_maps = [_prep_core_inputs(inputs, r) for r in range(N_CORES)]
    res = run_bass_kernel_spmd(nc, in_maps, list(range(N_CORES)),
                               trace=bool(os.environ.get("BASS_TRACE_RUN")))
    _CACHE["last_result"] = res

    out = np.zeros((S, B, O), np.float32)
    for r in range(N_CORES):
        o = res.results[r]["out"].reshape(S, BL, O)
        out[:, r * BL:(r + 1) * BL, :] = o
    return out
